# revision 1
# baseline (speedup 1.0000x reference)
"""Multi-head attention (B=4, S=2048, E=1024, H=16) on 8 TRN2 NeuronCores.

Sharding: batch x query-half data parallel -- core c handles batch c//2,
query rows [ (c%2)*1024 : (c%2+1)*1024 ].  Each core computes K/V for its
full batch (redundant KV projection, no collectives needed), runs all 16
heads of attention for its 1024 query rows, and the output projection.

Layout choices (all chosen so no on-chip transposes are needed):
  - x arrives pre-transposed from host as x^T [E, S] with the core's own
    query half first in the S order (attention is permutation-invariant
    along keys, so K/V use the same reordered S).
  - Q^T, K^T [d, s] produced by matmul(lhsT=W_slice, rhs=x^T).
  - scores computed transposed: S^T[k, q] = matmul(lhsT=K^T_tile, rhs=Q^T).
  - softmax denominator via a ones-column appended to V: the PV matmul
    (lhsT=V_aug [k,65], rhs=exp(S^T)) yields rows 0-63 = (P V)^T and
    row 64 = per-query sums, accumulated over k-tiles in PSUM for free.
  - output projection computed transposed: out^T = matmul(lhsT=W_out,
    rhs=SC^T); host transposes the [E, q] result when reassembling.

Compute dtype bf16 (weights/activations), fp32 PSUM accumulation, exp on
ScalarE in fp32 from PSUM.
"""

import sys

if "/opt/trn_rl_repo" not in sys.path:
    sys.path.insert(0, "/opt/trn_rl_repo")

import numpy as np
import ml_dtypes

B, S, E, H = 4, 2048, 1024, 16
HD = E // H  # 64
N_CORES = 8
QH = S // 2  # query rows per core (1024)
P = 128
ET = E // P  # 8 e-tiles
ST = S // P  # 16 s-tiles
QC = QH // 512  # 2 q chunks of 512

_BF16 = ml_dtypes.bfloat16

_cached = None  # (nc, run_fn)

DEBUG_DUMPS = False
REPEAT = 1
SKIP_NORM = False
NO_PBCAST = False
NO_EXTRACT = False
NO_RECIP = False
DUMP_P = False


def _build():
    import concourse.bass as bass
    import concourse.tile as tile
    import concourse.mybir as mybir
    from concourse import bacc

    dt = mybir.dt
    nc = bacc.Bacc("TRN2", target_bir_lowering=False, debug=False)

    xt_d = nc.dram_tensor("xt", [E, S], dt.bfloat16, kind="ExternalInput").ap()
    wq_d = nc.dram_tensor("wq", [E, E], dt.bfloat16, kind="ExternalInput").ap()
    wk_d = nc.dram_tensor("wk", [E, E], dt.bfloat16, kind="ExternalInput").ap()
    wv_d = nc.dram_tensor("wv", [E, E], dt.bfloat16, kind="ExternalInput").ap()
    wo_d = nc.dram_tensor("wo", [E, E], dt.bfloat16, kind="ExternalInput").ap()
    bq_d = nc.dram_tensor("bq", [P, ET], dt.float32, kind="ExternalInput").ap()
    bk_d = nc.dram_tensor("bk", [P, ET], dt.float32, kind="ExternalInput").ap()
    bv_d = nc.dram_tensor("bv", [1, E], dt.bfloat16, kind="ExternalInput").ap()
    bo_d = nc.dram_tensor("bo", [P, ET], dt.float32, kind="ExternalInput").ap()
    out_d = nc.dram_tensor("out", [E, QH], dt.float32, kind="ExternalOutput").ap()
    if DUMP_P:
        dp_d = nc.dram_tensor("dp", [H, ST, P, QH], dt.bfloat16, kind="ExternalOutput").ap()
    if DEBUG_DUMPS:
        dqt_d = nc.dram_tensor("dqt", [P, ET * QH], dt.bfloat16, kind="ExternalOutput").ap()
        dkt_d = nc.dram_tensor("dkt", [P, ET * S], dt.bfloat16, kind="ExternalOutput").ap()
        dva_d = nc.dram_tensor("dva", [P, ST * H * (HD + 1)], dt.bfloat16, kind="ExternalOutput").ap()
        dscb_d = nc.dram_tensor("dscb", [P, ET * QH], dt.bfloat16, kind="ExternalOutput").ap()

    SCALE = 1.0 / float(np.sqrt(HD))

    with tile.TileContext(nc) as tc:
        with (
            tc.tile_pool(name="const", bufs=1) as cpool,
            tc.tile_pool(name="acts", bufs=1) as apool,
            tc.tile_pool(name="work", bufs=2) as wpool,
            tc.tile_pool(name="norm", bufs=1) as npool,
            tc.tile_pool(name="norm2", bufs=2) as n2pool,
            tc.tile_pool(name="bcp", bufs=1) as bcpool,
            tc.tile_pool(name="wv2", bufs=2) as wv2pool,
        ):
          with (tc.For_i(0, REPEAT, 1) if REPEAT > 1 else __import__("contextlib").nullcontext()):
              xt = cpool.tile([P, ET, S], dt.bfloat16)
              wo = cpool.tile([P, ET, E], dt.bfloat16)
              bq = cpool.tile([P, ET], dt.float32)
              bk = cpool.tile([P, ET], dt.float32)
              bv = cpool.tile([1, E], dt.bfloat16)
              bo = cpool.tile([P, ET], dt.float32)
              ones1 = cpool.tile([1, P], dt.bfloat16)

              nc.sync.dma_start(bq[:], bq_d)
              nc.sync.dma_start(bk[:], bk_d)
              nc.sync.dma_start(bv[:], bv_d)
              nc.sync.dma_start(bo[:], bo_d)
              wvcs = []
              for c in range(2):
                  wvc = wv2pool.tile(
                      [P, ET, 512], dt.bfloat16, tag="wvc", name=f"wvc{c}"
                  )
                  nc.sync.dma_start(
                      wvc[:],
                      wv_d[:, c * 512 : (c + 1) * 512].rearrange(
                          "(eo p) c -> p eo c", p=P
                      ),
                  )
                  wvcs.append(wvc)
              for sx in range(4):
                  nc.sync.dma_start(
                      xt[:, :, sx * 512 : (sx + 1) * 512],
                      xt_d[:, sx * 512 : (sx + 1) * 512].rearrange(
                          "(eo p) s -> p eo s", p=P
                      ),
                  )
              nc.sync.dma_start(wo[:], wo_d.rearrange("(eo p) c -> p eo c", p=P))
              nc.gpsimd.memset(ones1[:], 1.0)

              qt = apool.tile([P, ET, QH], dt.bfloat16)   # Q^T + bq
              kt = apool.tile([P, ET, S], dt.bfloat16)    # K^T + bk
              va = apool.tile([P, ST, H, HD + 1], dt.bfloat16)  # V (+bias) | ones col
              scb = apool.tile([P, ET, QH], dt.bfloat16)  # normalized SC^T

              nc.vector.memset(va[:, :, :, HD : HD + 1], 1.0)

              # ---- interleaved phases 1+2: V projection first, then per
              # head-pair: its Q/K projection immediately followed by its
              # attention, all sharing one PSUM pool so the PE never drains.
              ph1 = tc.tile_pool(name="psA", bufs=2, space="PSUM")
              ph2 = tc.tile_pool(name="psB", bufs=4, space="PSUM")
              pspool = ph1.__enter__()   # tag "sc": [128,1024] 2-bank tiles
              ps4pool = ph2.__enter__()  # tag "sm": [128,512] 1-bank tiles

              # V projection: V[s, d] for all heads (+bias via K=1 matmul)
              for c in range(2):
                  wvc = wvcs[c]
                  for st in range(ST):
                      ps = ps4pool.tile([P, 512], dt.float32, tag="sm", name=f"psv{st}{c}")
                      for e in range(ET):
                          nc.tensor.matmul(
                              ps[:],
                              xt[:, e, st * P : (st + 1) * P],
                              wvc[:, e, :],
                              start=(e == 0),
                              stop=False,
                          )
                      nc.tensor.matmul(
                          ps[:],
                          ones1[0:1, :],
                          bv[0:1, c * 512 : (c + 1) * 512],
                          start=False,
                          stop=True,
                      )
                      nc.vector.tensor_copy(
                          va[:, st, c * 8 : (c + 1) * 8, 0:HD],
                          ps.rearrange("p (h d) -> p h d", d=HD),
                      )

              for t in range(ET):
                  # Q^T, K^T projection for this pair's head tile t
                  wqt = wpool.tile([P, ET, P], dt.bfloat16, tag="wt", name=f"wqt{t}")
                  nc.sync.dma_start(
                      wqt[:],
                      wq_d[:, t * P : (t + 1) * P].rearrange("(eo p) c -> p eo c", p=P),
                  )
                  psq = pspool.tile([P, 1024], dt.float32, tag="sc", name=f"psq{t}")
                  for c in range(QC):
                      for e in range(ET):
                          nc.tensor.matmul(
                              psq[:, c * 512 : (c + 1) * 512],
                              wqt[:, e, :],
                              xt[:, e, c * 512 : (c + 1) * 512],
                              start=(e == 0),
                              stop=(e == ET - 1),
                          )
                  nc.vector.tensor_scalar_add(qt[:, t, :], psq[:], bq[:, t : t + 1])
                  wkt = wpool.tile([P, ET, P], dt.bfloat16, tag="wt", name=f"wkt{t}")
                  nc.sync.dma_start(
                      wkt[:],
                      wk_d[:, t * P : (t + 1) * P].rearrange("(eo p) c -> p eo c", p=P),
                  )
                  for ck in range(2):
                      psk = pspool.tile([P, 1024], dt.float32, tag="sc", name=f"psk{t}{ck}")
                      for c in range(2):
                          for e in range(ET):
                              nc.tensor.matmul(
                                  psk[:, c * 512 : (c + 1) * 512],
                                  wkt[:, e, :],
                                  xt[:, e, (ck * 2 + c) * 512 : (ck * 2 + c + 1) * 512],
                                  start=(e == 0),
                                  stop=(e == ET - 1),
                              )
                      nc.vector.tensor_scalar_add(
                          kt[:, t, ck * 1024 : (ck + 1) * 1024], psk[:], bk[:, t : t + 1]
                      )

                  # attention for heads (2t, 2t+1), interleaved in the PE array
                  pv = [
                      ps4pool.tile([P, 512], dt.float32, tag="sm", name=f"pv{t}{i}")
                      for i in range(4)  # [even c0, even c1, odd c0, odd c1]
                  ]
                  for j in range(ST):
                      for c in range(QC):
                          sc = pspool.tile(
                              [P, 1024], dt.float32, tag="sc", name=f"sc{t}{j}{c}"
                          )
                          nc.tensor.matmul(
                              sc[:, 0:512],
                              kt[0:64, t, j * P : (j + 1) * P],
                              qt[0:64, t, c * 512 : (c + 1) * 512],
                              start=True,
                              stop=True,
                          )
                          nc.tensor.matmul(
                              sc[:, 512:1024],
                              kt[64:128, t, j * P : (j + 1) * P],
                              qt[64:128, t, c * 512 : (c + 1) * 512],
                              start=True,
                              stop=True,
                          )
                          p = wpool.tile([P, 1024], dt.bfloat16, tag="p")
                          nc.scalar.activation(
                              p[:], sc[:], mybir.ActivationFunctionType.Exp, scale=SCALE
                          )
                          if DUMP_P:
                              nc.sync.dma_start(dp_d[2 * t, j, :, c * 512 : (c + 1) * 512], p[:, 0:512])
                              nc.sync.dma_start(dp_d[2 * t + 1, j, :, c * 512 : (c + 1) * 512], p[:, 512:1024])
                          nc.tensor.matmul(
                              pv[c][0 : HD + 1, :],
                              va[:, j, 2 * t, :],
                              p[:, 0:512],
                              start=(j == 0),
                              stop=(j == ST - 1),
                          )
                          nc.tensor.matmul(
                              pv[2 + c][0 : HD + 1, :],
                              va[:, j, 2 * t + 1, :],
                              p[:, 512:1024],
                              start=(j == 0),
                              stop=(j == ST - 1),
                          )
                  # fast raw evacuation releases the PSUM slots; normalize after
                  for half in range(2):
                      h = 2 * t + half
                      hp = half * 64
                      pvr = n2pool.tile([64, QH], dt.bfloat16, tag="pvr", name=f"pvr{h}")
                      srow_t = n2pool.tile([1, QH], dt.float32, tag="srow", name=f"sr{h}")
                      scr_t = npool.tile([1, QH], dt.float32, tag="scr", name=f"sx{h}")
                      rrow_t = npool.tile([1, QH], dt.float32, tag="rrow", name=f"rr{h}")
                      srow, scr, rrow = srow_t[0:1, :], scr_t[0:1, :], rrow_t[0:1, :]
                      for c in range(QC):
                          nc.vector.tensor_copy(
                              pvr[:, c * 512 : (c + 1) * 512],
                              pv[2 * half + c][0:HD, :],
                          )
                          nc.vector.tensor_copy(
                              srow[0:1, c * 512 : (c + 1) * 512],
                              pv[2 * half + c][HD : HD + 1, :],
                          )
                      nc.vector.reciprocal_approx_accurate(rrow, srow, scr)
                      bc = bcpool.tile([64, QH], dt.float32, tag="bc", name=f"bc{h}")
                      nc.gpsimd.partition_broadcast(bc[:], rrow[0:1, :])
                      nc.vector.tensor_tensor(
                          scb[hp : hp + HD, t, :],
                          pvr[:],
                          bc[:],
                          mybir.AluOpType.mult,
                      )
              # ---- phase 3: output projection (transposed) + bias
              # reuses the "sc" psum tag so its slots rotate in as exp frees them
              for t2 in range(ET):
                  ps = pspool.tile([P, 1024], dt.float32, tag="sc", name=f"pso{t2}")
                  for c in range(QC):
                      for e in range(ET):
                          nc.tensor.matmul(
                              ps[:, c * 512 : (c + 1) * 512],
                              wo[:, e, t2 * P : (t2 + 1) * P],
                              scb[:, e, c * 512 : (c + 1) * 512],
                              start=(e == 0),
                              stop=(e == ET - 1),
                          )
                  ot = wpool.tile([P, QH], dt.float32, tag="ot", name=f"ot{t2}")
                  nc.vector.tensor_scalar_add(ot[:], ps[:], bo[:, t2 : t2 + 1])
                  nc.sync.dma_start(out_d[t2 * P : (t2 + 1) * P, :], ot[:])
              ph2.__exit__(None, None, None)
              ph1.__exit__(None, None, None)

    nc.compile()
    return nc


def _prep_inputs(x, W_qkv, b_qkv, W_out, b_out):
    """Host-side sharding + layout prep. Returns per-core input maps."""
    w = W_qkv.reshape(E, H, 3, HD)
    wq = np.ascontiguousarray(w[:, :, 0, :].reshape(E, E)).astype(_BF16)
    wk = np.ascontiguousarray(w[:, :, 1, :].reshape(E, E)).astype(_BF16)
    wv = np.ascontiguousarray(w[:, :, 2, :].reshape(E, E)).astype(_BF16)
    wo = W_out.astype(_BF16)
    b3 = b_qkv.reshape(H, 3, HD)
    bq = np.ascontiguousarray(b3[:, 0, :].reshape(ET, P).T).astype(np.float32)
    bk = np.ascontiguousarray(b3[:, 1, :].reshape(ET, P).T).astype(np.float32)
    bv = np.ascontiguousarray(b3[:, 2, :].reshape(1, E)).astype(_BF16)
    bo = np.ascontiguousarray(b_out.reshape(ET, P).T).astype(np.float32)

    in_maps = []
    for core in range(N_CORES):
        b, half = core // 2, core % 2
        xb = x[b]  # [S, E]
        order = np.r_[half * QH : (half + 1) * QH, (1 - half) * QH : (2 - half) * QH]
        xt = np.ascontiguousarray(xb[order].T).astype(_BF16)  # [E, S], own q first
        in_maps.append(
            {
                "xt": xt,
                "wq": wq,
                "wk": wk,
                "wv": wv,
                "wo": wo,
                "bq": bq,
                "bk": bk,
                "bv": bv,
                "bo": bo,
            }
        )
    return in_maps


def run_raw(x, W_qkv, b_qkv, W_out, b_out, trace=False, **kw):
    """Run on hardware; returns (full_output [B,S,E] f32, BassKernelResults)."""
    global _cached
    from concourse.bass_utils import run_bass_kernel_spmd

    if _cached is None:
        _cached = _build()
    nc = _cached
    in_maps = _prep_inputs(x, W_qkv, b_qkv, W_out, b_out)
    res = run_bass_kernel_spmd(
        nc, in_maps, core_ids=list(range(N_CORES)), trace=trace, **kw
    )
    out = np.empty((B, S, E), dtype=np.float32)
    for core in range(N_CORES):
        b, half = core // 2, core % 2
        out[b, half * QH : (half + 1) * QH, :] = np.asarray(
            res.results[core]["out"]
        ).T
    return out, res


def kernel(x, W_qkv, b_qkv, W_out, b_out):
    out, _ = run_raw(x, W_qkv, b_qkv, W_out, b_out, trace=False)
    return out



# revision 2
# speedup vs baseline: 1.4578x; 1.4578x over previous
"""Multi-head attention (B=4, S=2048, E=1024, H=16) on 8 TRN2 NeuronCores.

Sharding: batch x head-half tensor parallel -- core c handles batch c//2,
heads (c%2)*8 .. (c%2)*8+8, for ALL 2048 query rows.  QKV projections are
column-split by head (no redundant work); the output projection is
row-split: each core produces a partial out^T [E, S] and the HOST sums the
two partials per batch (plus the fused bias row b_out + b_v @ W_out).

Key layout/efficiency choices (cost model: matmul time = streamed columns):
  - x arrives as x^T [E, S]; Q^T/K^T [d, s] via matmul(lhsT=W_slice, rhs=x^T).
  - scores transposed: S^T[k, q] = matmul(lhsT=K^T tile, rhs=Q^T), streamed
    over q; exp on ScalarE from PSUM.
  - PV computed NON-transposed: PV[q, d] = matmul(lhsT=p[k, q-tile],
    rhs=V_aug[k, 65]) -- streams only 65 columns per (q-tile, k-tile), half
    the PE work of the transposed form.  The ones column of V_aug yields the
    softmax denominators in column 64; normalization is then a per-partition
    tensor_scalar multiply (reciprocal of the strided denominator columns).
  - normalized SC[q, d] tiles are transposed back to SC^T via PE transpose
    (identity matmul) for the output projection, which streams q.
  - K bias is dropped entirely (softmax-invariant); V bias and out bias are
    folded into a host-side bias row (b_out + b_v @ W_out) added after the
    partial sum; only the Q bias is applied in-kernel.
  - instruction stream interleaves "filler" projection matmuls for the next
    head-pair into each attention j-loop so the PE never idles while the
    Activation engine (exp, the attention-phase pacer) catches up.

Compute bf16, fp32 PSUM accumulation.
"""

import sys

if "/opt/trn_rl_repo" not in sys.path:
    sys.path.insert(0, "/opt/trn_rl_repo")

import numpy as np
import ml_dtypes

B, S, E, H = 4, 2048, 1024, 16
HD = E // H  # 64
N_CORES = 8
P = 128
ET = E // P          # 8 e-tiles (contraction tiles for projections)
ST = S // P          # 16 s-tiles (k-tiles)
HPC = 8              # heads per core
DT = HPC * HD // P   # 4 d-tiles per core (2 heads per tile)
NCH = 4              # q-chunks of 512
CH = 512

_BF16 = ml_dtypes.bfloat16

_cached = None

SCALE = 1.0 / float(np.sqrt(HD))


def _build():
    import concourse.bass as bass
    import concourse.tile as tile
    import concourse.mybir as mybir
    from concourse import bacc
    from concourse.masks import make_identity
    from collections import deque

    dt = mybir.dt
    nc = bacc.Bacc("TRN2", target_bir_lowering=False, debug=False)

    xt_d = nc.dram_tensor("xt", [E, S], dt.bfloat16, kind="ExternalInput").ap()
    wq_d = nc.dram_tensor("wq", [P, ET, 512], dt.bfloat16, kind="ExternalInput").ap()
    wk_d = nc.dram_tensor("wk", [P, ET, 512], dt.bfloat16, kind="ExternalInput").ap()
    wv_d = nc.dram_tensor("wv", [P, ET, 512], dt.bfloat16, kind="ExternalInput").ap()
    wo_d = nc.dram_tensor("wo", [P, DT, E], dt.bfloat16, kind="ExternalInput").ap()
    bq_d = nc.dram_tensor("bq", [P, DT], dt.float32, kind="ExternalInput").ap()
    out_d = nc.dram_tensor("out", [E, S], dt.float32, kind="ExternalOutput").ap()

    with tile.TileContext(nc) as tc:
        with (
            tc.tile_pool(name="const", bufs=1) as cpool,
            tc.tile_pool(name="acts", bufs=1) as apool,
            tc.tile_pool(name="p", bufs=3) as ppool,
            tc.tile_pool(name="scn", bufs=10) as scnpool,
            tc.tile_pool(name="rr", bufs=4) as rrpool,
            tc.tile_pool(name="ev", bufs=3) as evpool,
            tc.tile_pool(name="psc", bufs=2, space="PSUM") as psc,
            tc.tile_pool(name="ppv", bufs=2, space="PSUM") as ppv,
            tc.tile_pool(name="pproj", bufs=2, space="PSUM") as pproj,
        ):
            # ---------------- constants / DMAs ----------------
            xt = cpool.tile([P, ET, S], dt.bfloat16)
            wq = cpool.tile([P, ET, 512], dt.bfloat16)
            wk = cpool.tile([P, ET, 512], dt.bfloat16)
            wv = cpool.tile([P, ET, 512], dt.bfloat16)
            wo = cpool.tile([P, DT, E], dt.bfloat16)
            bq = cpool.tile([P, DT], dt.float32)
            ident = cpool.tile([P, P], dt.bfloat16)

            nc.sync.dma_start(wq[:], wq_d)
            for sx in range(4):
                nc.sync.dma_start(
                    xt[:, :, sx * CH : (sx + 1) * CH],
                    xt_d[:, sx * CH : (sx + 1) * CH].rearrange(
                        "(eo p) s -> p eo s", p=P
                    ),
                )
            nc.sync.dma_start(wk[:], wk_d)
            nc.sync.dma_start(wv[:], wv_d)
            nc.sync.dma_start(bq[:], bq_d)
            nc.sync.dma_start(wo[:], wo_d)
            make_identity(nc, ident[:])

            qt = apool.tile([P, DT, S], dt.bfloat16)   # Q^T + bq
            kt = apool.tile([P, DT, S], dt.bfloat16)   # K^T (no bias)
            va = apool.tile([P, ST, HPC, HD + 1], dt.bfloat16)  # V | ones col
            sct = apool.tile([P, DT, S], dt.bfloat16)  # normalized SC^T

            nc.gpsimd.memset(va[:, :, :, HD : HD + 1], 1.0)

            # ---------------- filler machinery ----------------
            # Each filler is a callable emitting ONE PE matmul (evictions are
            # emitted inline by the last matmul of a chunk; they run on DVE
            # and do not consume PE time).
            fillers = deque()

            def q_chunk(t, c):
                # Q^T chunk: psum [P, 512] accumulating over ET e-tiles
                state = {}

                def mk(e):
                    def emit():
                        if e == 0:
                            state["ps"] = pproj.tile(
                                [P, CH], dt.float32, tag="proj", name=f"q{t}{c}"
                            )
                        nc.tensor.matmul(
                            state["ps"][:],
                            wq[:, e, t * P : (t + 1) * P],
                            xt[:, e, c * CH : (c + 1) * CH],
                            start=(e == 0),
                            stop=(e == ET - 1),
                        )
                        if e == ET - 1:
                            nc.vector.tensor_scalar_add(
                                qt[:, t, c * CH : (c + 1) * CH],
                                state["ps"][:],
                                bq[:, t : t + 1],
                            )

                    return emit

                return [mk(e) for e in range(ET)]

            def k_chunk(t, c):
                state = {}

                def mk(e):
                    def emit():
                        if e == 0:
                            state["ps"] = pproj.tile(
                                [P, CH], dt.float32, tag="proj", name=f"k{t}{c}"
                            )
                        nc.tensor.matmul(
                            state["ps"][:],
                            wk[:, e, t * P : (t + 1) * P],
                            xt[:, e, c * CH : (c + 1) * CH],
                            start=(e == 0),
                            stop=(e == ET - 1),
                        )
                        if e == ET - 1:
                            nc.vector.tensor_copy(
                                kt[:, t, c * CH : (c + 1) * CH], state["ps"][:]
                            )

                    return emit

                return [mk(e) for e in range(ET)]

            def v_chunk(t, st):
                # V rows for s-tile st, heads 2t, 2t+1: psum [P, 128]
                state = {}

                def mk(e):
                    def emit():
                        if e == 0:
                            state["ps"] = pproj.tile(
                                [P, 2 * HD], dt.float32, tag="proj", name=f"v{t}{st}"
                            )
                        nc.tensor.matmul(
                            state["ps"][:],
                            xt[:, e, st * P : (st + 1) * P],
                            wv[:, e, t * P : (t + 1) * P],
                            start=(e == 0),
                            stop=(e == ET - 1),
                        )
                        if e == ET - 1:
                            nc.vector.tensor_copy(
                                va[:, st, 2 * t : 2 * t + 2, 0:HD],
                                state["ps"].rearrange("p (h d) -> p h d", d=HD),
                            )

                    return emit

                return [mk(e) for e in range(ET)]

            def out_chunk(t2, qr):
                # out^T chunk [P e', 512 q] contracting SC dims (4 ct tiles)
                state = {}

                def mk(ct):
                    def emit():
                        if ct == 0:
                            state["ps"] = pproj.tile(
                                [P, CH], dt.float32, tag="proj", name=f"o{t2}{qr}"
                            )
                        nc.tensor.matmul(
                            state["ps"][:],
                            wo[:, ct, t2 * P : (t2 + 1) * P],
                            sct[:, ct, qr * CH : (qr + 1) * CH],
                            start=(ct == 0),
                            stop=(ct == DT - 1),
                        )
                        if ct == DT - 1:
                            ev = evpool.tile(
                                [P, CH], dt.float32, tag="ev", name=f"ev{t2}{qr}"
                            )
                            nc.vector.tensor_copy(ev[:], state["ps"][:])
                            nc.sync.dma_start(
                                out_d[t2 * P : (t2 + 1) * P, qr * CH : (qr + 1) * CH],
                                ev[:],
                            )

                    return emit

                return [mk(ct) for ct in range(DT)]

            def pump(n):
                for _ in range(n):
                    if not fillers:
                        break
                    fillers.popleft()()

            # ---------------- preamble: Q(0), K(0), V(0) ----------------
            for c in range(NCH):
                for f in q_chunk(0, c):
                    f()
                for f in k_chunk(0, c):
                    f()
            for st in range(ST):
                for f in v_chunk(0, st):
                    f()

            # ---------------- main pair loop ----------------
            for t in range(DT):
                # queue fillers for the next pair (or the out projection)
                if t < DT - 1:
                    for st in range(ST):
                        fillers.extend(v_chunk(t + 1, st))
                    for c in range(NCH):
                        fillers.extend(q_chunk(t + 1, c))
                        fillers.extend(k_chunk(t + 1, c))

                for c in range(NCH):
                    if t == DT - 1 and c >= 2:
                        # SC^T for q-range c-2 is complete across all pairs
                        qr = c - 2
                        for t2 in range(ET):
                            fillers.extend(out_chunk(t2, qr))
                    pvh = [
                        ppv.tile([P, 4 * (HD + 1)], dt.float32, tag="pv", name=f"pv{t}{c}{i}")
                        for i in range(2)
                    ]
                    for j in range(ST):
                        sc = psc.tile([P, 1024], dt.float32, tag="sc", name=f"sc{t}{c}{j}")
                        nc.tensor.matmul(
                            sc[:, 0:512],
                            kt[0:HD, t, j * P : (j + 1) * P],
                            qt[0:HD, t, c * CH : (c + 1) * CH],
                            start=True,
                            stop=True,
                        )
                        nc.tensor.matmul(
                            sc[:, 512:1024],
                            kt[HD:P, t, j * P : (j + 1) * P],
                            qt[HD:P, t, c * CH : (c + 1) * CH],
                            start=True,
                            stop=True,
                        )
                        p = ppool.tile([P, 1024], dt.bfloat16, tag="p")
                        nc.scalar.activation(
                            p[:], sc[:], mybir.ActivationFunctionType.Exp, scale=SCALE
                        )
                        for h_i in range(2):
                            for qs in range(4):
                                nc.tensor.matmul(
                                    pvh[h_i][:, qs * (HD + 1) : (qs + 1) * (HD + 1)],
                                    p[:, h_i * 512 + qs * P : h_i * 512 + (qs + 1) * P],
                                    va[:, j, 2 * t + h_i, :],
                                    start=(j == 0 and qs == 0),
                                    stop=(j == ST - 1 and qs == 3),
                                )
                        pump(3)
                    # normalize + transpose this (t, c)
                    scns = [[None] * 4 for _ in range(2)]
                    for h_i in range(2):
                        rr = rrpool.tile([P, 4], dt.float32, tag="rr", name=f"rr{t}{c}{h_i}")
                        nc.vector.reciprocal(
                            rr[:], pvh[h_i][:, HD : 4 * (HD + 1) : HD + 1]
                        )
                        for qs in range(4):
                            scn = scnpool.tile([P, HD], dt.bfloat16, tag="scn")
                            nc.vector.tensor_scalar_mul(
                                scn[:],
                                pvh[h_i][:, qs * (HD + 1) : qs * (HD + 1) + HD],
                                rr[:, qs : qs + 1],
                            )
                            scns[h_i][qs] = scn
                    sct_ps = ppv.tile([P, CH], dt.bfloat16, tag="pv", name=f"tp{t}{c}")
                    for h_i in range(2):
                        for qs in range(4):
                            nc.tensor.transpose(
                                sct_ps[
                                    h_i * HD : (h_i + 1) * HD,
                                    qs * P : (qs + 1) * P,
                                ],
                                scns[h_i][qs][:],
                                ident[:],
                            )
                    nc.vector.tensor_copy(
                        sct[:, t, c * CH : (c + 1) * CH], sct_ps[:]
                    )
            # ---------------- drain remaining fillers (out proj tail) ----
            for t2 in range(ET):
                fillers.extend(out_chunk(t2, 2))
            for t2 in range(ET):
                fillers.extend(out_chunk(t2, 3))
            pump(len(fillers))

    nc.compile()
    return nc


def _prep_inputs(x, W_qkv, b_qkv, W_out, b_out):
    """Host-side sharding + layout prep. Returns per-core input maps."""
    w = W_qkv.reshape(E, H, 3, HD)
    wq_f = np.ascontiguousarray(w[:, :, 0, :].reshape(E, E)).astype(_BF16)
    wk_f = np.ascontiguousarray(w[:, :, 1, :].reshape(E, E)).astype(_BF16)
    wv_f = np.ascontiguousarray(w[:, :, 2, :].reshape(E, E)).astype(_BF16)
    b3 = b_qkv.reshape(H, 3, HD)
    bq_f = np.ascontiguousarray(b3[:, 0, :].reshape(E)).astype(np.float32)

    in_maps = []
    xts = [np.ascontiguousarray(x[b].T).astype(_BF16) for b in range(B)]
    halves = []
    for hh in range(2):
        cols = slice(hh * 512, (hh + 1) * 512)
        wq = np.ascontiguousarray(
            wq_f[:, cols].reshape(ET, P, 512).transpose(1, 0, 2)
        )
        wk = np.ascontiguousarray(
            wk_f[:, cols].reshape(ET, P, 512).transpose(1, 0, 2)
        )
        wv = np.ascontiguousarray(
            wv_f[:, cols].reshape(ET, P, 512).transpose(1, 0, 2)
        )
        wo = np.ascontiguousarray(
            W_out[hh * 512 : (hh + 1) * 512, :]
            .reshape(DT, P, E)
            .transpose(1, 0, 2)
        ).astype(_BF16)
        bq = np.ascontiguousarray(bq_f[cols].reshape(DT, P).T)
        halves.append({"wq": wq, "wk": wk, "wv": wv, "wo": wo, "bq": bq})
    for core in range(N_CORES):
        b, hh = core // 2, core % 2
        m = {"xt": xts[b]}
        m.update(halves[hh])
        in_maps.append(m)
    return in_maps


def run_raw(x, W_qkv, b_qkv, W_out, b_out, trace=False, **kw):
    """Run on hardware; returns (full_output [B,S,E] f32, BassKernelResults)."""
    global _cached
    from concourse.bass_utils import run_bass_kernel_spmd

    if _cached is None:
        _cached = _build()
    nc = _cached
    in_maps = _prep_inputs(x, W_qkv, b_qkv, W_out, b_out)
    res = run_bass_kernel_spmd(
        nc, in_maps, core_ids=list(range(N_CORES)), trace=trace, **kw
    )
    # host: sum row-split partials, transpose, add fused bias row
    bv_f = np.asarray(b_qkv, dtype=np.float64).reshape(H, 3, HD)[:, 2, :].reshape(E)
    bias_row = (
        np.asarray(b_out, dtype=np.float64)
        + bv_f @ np.asarray(W_out, dtype=np.float64)
    ).astype(np.float32)
    out = np.empty((B, S, E), dtype=np.float32)
    for b in range(B):
        o0 = np.asarray(res.results[2 * b]["out"])
        o1 = np.asarray(res.results[2 * b + 1]["out"])
        out[b] = (o0 + o1).T + bias_row
    return out, res


def kernel(x, W_qkv, b_qkv, W_out, b_out):
    out, _ = run_raw(x, W_qkv, b_qkv, W_out, b_out)
    return out


# revision 8
# speedup vs baseline: 1.5692x; 1.0764x over previous
"""Multi-head attention (B=4, S=2048, E=1024, H=16) on 8 TRN2 NeuronCores.

Sharding: batch x head-half tensor parallel -- core c handles batch c//2,
heads (c%2)*8 .. (c%2)*8+8, for ALL 2048 query rows.  QKV projections are
column-split by head (no redundant work); the output projection is
row-split: each core produces a partial out^T [E, S] (bf16) and the HOST
sums the two partials per batch in fp32 (plus the fused bias row
b_out + b_v @ W_out).

Key layout/efficiency choices (cost model: matmul time = streamed columns):
  - x arrives as x^T [E, S]; Q^T/K^T [d, s] via matmul(lhsT=W_slice, rhs=x^T).
  - scores transposed: S^T[k, q] = matmul(lhsT=K^T tile, rhs=Q^T), streamed
    over q; exp on ScalarE from PSUM (the attention-phase pacer).
  - PV computed NON-transposed: PV[q, d] = matmul(lhsT=p[k, q-tile],
    rhs=V_aug[k, 65]) -- streams only 65 columns per (q-tile, k-tile), half
    the PE work of the transposed form.  The ones column of V_aug yields the
    softmax denominators in column 64; normalization is a per-partition
    tensor_scalar multiply with the reciprocal of the strided denominators.
  - normalized SC[q, d] tiles are transposed back to SC^T via PE transpose
    (identity matmul) for the output projection, which streams q.
  - K bias is dropped entirely (softmax-invariant); V bias and out bias are
    folded into a host-side bias row (b_out + b_v @ W_out) added after the
    partial sum; only the Q bias is applied in-kernel.
  - score matmuls are software-pipelined one j-step ahead (across window
    boundaries too) so the Activation engine never waits; "filler"
    projection / output matmuls are column-budget pumped into each j
    iteration, deadline-ordered against the progressive x^T DMA arrival,
    so the PE never idles while Activation catches up.

Compute bf16, fp32 PSUM accumulation.
"""

import sys

if "/opt/trn_rl_repo" not in sys.path:
    sys.path.insert(0, "/opt/trn_rl_repo")

import numpy as np
import ml_dtypes

B, S, E, H = 4, 2048, 1024, 16
HD = E // H  # 64
N_CORES = 8
P = 128
ET = E // P          # 8 e-tiles (contraction tiles for projections)
ST = S // P          # 16 s-tiles (k-tiles)
HPC = 8              # heads per core
DT = HPC * HD // P   # 4 d-tiles per core (2 heads per tile)
NCH = 4              # q-chunks of 512
CH = 512

_BF16 = ml_dtypes.bfloat16

_cached = None

SCALE = 1.0 / float(np.sqrt(HD))


def _build():
    import concourse.bass as bass
    import concourse.tile as tile
    import concourse.mybir as mybir
    from concourse import bacc
    from concourse.masks import make_identity
    from collections import deque

    dt = mybir.dt
    nc = bacc.Bacc("TRN2", target_bir_lowering=False, debug=False)

    xt_d = nc.dram_tensor("xt", [E, S], dt.bfloat16, kind="ExternalInput").ap()
    wq_d = nc.dram_tensor("wq", [P, DT, ET, P], dt.bfloat16, kind="ExternalInput").ap()
    wk_d = nc.dram_tensor("wk", [P, DT, ET, P], dt.bfloat16, kind="ExternalInput").ap()
    wv_d = nc.dram_tensor("wv", [P, DT, ET, P], dt.bfloat16, kind="ExternalInput").ap()
    wo_d = nc.dram_tensor("wo", [P, DT, E], dt.bfloat16, kind="ExternalInput").ap()
    bq_d = nc.dram_tensor("bq", [P, DT], dt.float32, kind="ExternalInput").ap()
    out_d = nc.dram_tensor("out", [E, S], dt.bfloat16, kind="ExternalOutput").ap()

    with tile.TileContext(nc) as tc:
        with (
            tc.tile_pool(name="const", bufs=1) as cpool,
            tc.tile_pool(name="acts", bufs=1) as apool,
            tc.tile_pool(name="p", bufs=6) as ppool,
            tc.tile_pool(name="scn", bufs=10) as scnpool,
            tc.tile_pool(name="rr", bufs=4) as rrpool,
            tc.tile_pool(name="psc", bufs=2, space="PSUM") as psc,
            tc.tile_pool(name="ppv", bufs=2, space="PSUM") as ppv,
            tc.tile_pool(name="pproj", bufs=2, space="PSUM") as pproj,
        ):
            # ---------------- constants / DMAs ----------------
            xt = cpool.tile([P, ET, S], dt.bfloat16)
            wq = cpool.tile([P, DT, ET, P], dt.bfloat16)
            wk = cpool.tile([P, DT, ET, P], dt.bfloat16)
            wv = cpool.tile([P, DT, ET, P], dt.bfloat16)
            wo = cpool.tile([P, DT, E], dt.bfloat16)
            bq = cpool.tile([P, DT], dt.float32)
            ident = cpool.tile([P, P], dt.bfloat16)

            # DMA schedule: the startup critical path is this serial chain,
            # so pair-0's weight slices (contiguous in the pair-major DRAM
            # layout, ~0.7us each) and the first half of xt chunk 0 come
            # first; the rest follows in deadline order; wo is needed last.
            def xt_cols_dma(lo, hi):
                nc.sync.dma_start(
                    xt[:, :, lo:hi],
                    xt_d[:, lo:hi].rearrange("(eo p) s -> p eo s", p=P),
                )

            nc.sync.dma_start(wq[:, 0], wq_d[:, 0])
            xt_cols_dma(0, 256)
            nc.sync.dma_start(wk[:, 0], wk_d[:, 0])
            xt_cols_dma(256, 512)
            nc.sync.dma_start(wv[:, 0], wv_d[:, 0])
            nc.sync.dma_start(bq[:], bq_d)
            for sx in range(1, 4):
                xt_cols_dma(sx * CH, (sx + 1) * CH)
            nc.sync.dma_start(wq[:, 1:], wq_d[:, 1:])
            nc.sync.dma_start(wk[:, 1:], wk_d[:, 1:])
            nc.sync.dma_start(wv[:, 1:], wv_d[:, 1:])
            nc.sync.dma_start(wo[:], wo_d)
            make_identity(nc, ident[:])

            qt = apool.tile([P, DT, S], dt.bfloat16)   # Q^T + bq
            kt = apool.tile([P, DT, S], dt.bfloat16)   # K^T (no bias)
            va = apool.tile([P, ST, HPC, HD + 1], dt.bfloat16)  # V | ones col
            sct = apool.tile([P, DT, S], dt.bfloat16)  # normalized SC^T
            ev = apool.tile([P, ET, S], dt.bfloat16)   # staged out^T partial

            nc.gpsimd.memset(va[:, :, :, HD : HD + 1], 1.0)

            # ---------------- filler machinery ----------------
            # Fillers are (cols, callable, earliest) triples, each emitting
            # ONE PE matmul (chunk evictions are emitted inline by the
            # chunk's last matmul; they run on DVE and cost no PE time).
            # pump(budget, now) pops fillers until the column budget is
            # spent, but never emits a filler before its `earliest`
            # window*16+j position -- this keeps DMA-gated matmuls from
            # entering the in-order PE queue early and head-of-line blocking
            # the attention stream behind them.
            fillers = deque()
            done_keys = set()

            def _pop_one():
                cols, fn, _e, key = fillers.popleft()
                fn()
                if key is not None:
                    done_keys.add(key)
                return cols

            def need(*keys):
                # correctness guard: force-emit queued fillers (in order,
                # ignoring gates/budgets) until every producer key a consumer
                # is about to read has been emitted.  Emission order is what
                # the tile framework builds dependencies from.
                for key in keys:
                    while key not in done_keys:
                        assert fillers, f"missing producer for {key}"
                        _pop_one()

            def q_chunk(t, c, earliest=0):
                state = {}
                key = ("q", t, c)

                def mk(e):
                    def emit():
                        if e == 0:
                            state["ps"] = pproj.tile(
                                [P, CH], dt.float32, tag="proj", name=f"q{t}{c}"
                            )
                        nc.tensor.matmul(
                            state["ps"][:],
                            wq[:, t, e, :],
                            xt[:, e, c * CH : (c + 1) * CH],
                            start=(e == 0),
                            stop=(e == ET - 1),
                        )
                        if e == ET - 1:
                            nc.vector.tensor_scalar_add(
                                qt[:, t, c * CH : (c + 1) * CH],
                                state["ps"][:],
                                bq[:, t : t + 1],
                            )

                    return emit

                return [(CH, mk(e), earliest, key if e == ET - 1 else None) for e in range(ET)]

            def k_chunk(t, c, earliest=0):
                state = {}
                key = ("k", t, c)

                def mk(e):
                    def emit():
                        if e == 0:
                            state["ps"] = pproj.tile(
                                [P, CH], dt.float32, tag="proj", name=f"k{t}{c}"
                            )
                        nc.tensor.matmul(
                            state["ps"][:],
                            wk[:, t, e, :],
                            xt[:, e, c * CH : (c + 1) * CH],
                            start=(e == 0),
                            stop=(e == ET - 1),
                        )
                        if e == ET - 1:
                            nc.vector.tensor_copy(
                                kt[:, t, c * CH : (c + 1) * CH], state["ps"][:]
                            )

                    return emit

                return [(CH, mk(e), earliest, key if e == ET - 1 else None) for e in range(ET)]

            def k_tile(t, j, earliest=0):
                # fine-grained K chunk: a single 128-wide k-tile
                state = {}
                key = ("ktile", t, j)

                def mk(e):
                    def emit():
                        if e == 0:
                            state["ps"] = pproj.tile(
                                [P, P], dt.float32, tag="proj", name=f"kt{t}_{j}"
                            )
                        nc.tensor.matmul(
                            state["ps"][:],
                            wk[:, t, e, :],
                            xt[:, e, j * P : (j + 1) * P],
                            start=(e == 0),
                            stop=(e == ET - 1),
                        )
                        if e == ET - 1:
                            nc.vector.tensor_copy(
                                kt[:, t, j * P : (j + 1) * P], state["ps"][:]
                            )

                    return emit

                return [(P, mk(e), earliest, key if e == ET - 1 else None) for e in range(ET)]

            def q_half(t, c, half):
                # 256-wide Q chunk (startup granularity, xt arrives in halves)
                lo = c * CH + half * 256
                state = {}
                key = ("qhalf", t, c, half)

                def mk(e):
                    def emit():
                        if e == 0:
                            state["ps"] = pproj.tile(
                                [P, 256], dt.float32, tag="proj", name=f"qh{t}{c}{half}"
                            )
                        nc.tensor.matmul(
                            state["ps"][:],
                            wq[:, t, e, :],
                            xt[:, e, lo : lo + 256],
                            start=(e == 0),
                            stop=(e == ET - 1),
                        )
                        if e == ET - 1:
                            nc.vector.tensor_scalar_add(
                                qt[:, t, lo : lo + 256],
                                state["ps"][:],
                                bq[:, t : t + 1],
                            )

                    return emit

                return [(256, mk(e), 0, key if e == ET - 1 else None) for e in range(ET)]

            def v_chunk(t, st, earliest=0):
                state = {}
                key = ("v", t, st)

                def mk(e):
                    def emit():
                        if e == 0:
                            state["ps"] = pproj.tile(
                                [P, 2 * HD], dt.float32, tag="proj", name=f"v{t}{st}"
                            )
                        nc.tensor.matmul(
                            state["ps"][:],
                            xt[:, e, st * P : (st + 1) * P],
                            wv[:, t, e, :],
                            start=(e == 0),
                            stop=(e == ET - 1),
                        )
                        if e == ET - 1:
                            nc.vector.tensor_copy(
                                va[:, st, 2 * t : 2 * t + 2, 0:HD],
                                state["ps"].rearrange("p (h d) -> p h d", d=HD),
                            )

                    return emit

                return [(2 * HD, mk(e), earliest, key if e == ET - 1 else None) for e in range(ET)]

            def out_chunk(t2, qr, earliest=0):
                state = {}

                def mk(ct):
                    def emit():
                        if ct == 0:
                            state["ps"] = pproj.tile(
                                [P, CH], dt.float32, tag="proj", name=f"o{t2}{qr}"
                            )
                        nc.tensor.matmul(
                            state["ps"][:],
                            wo[:, ct, t2 * P : (t2 + 1) * P],
                            sct[:, ct, qr * CH : (qr + 1) * CH],
                            start=(ct == 0),
                            stop=(ct == DT - 1),
                        )
                        if ct == DT - 1:
                            nc.vector.tensor_copy(
                                ev[:, t2, qr * CH : (qr + 1) * CH], state["ps"][:]
                            )

                    return emit

                return [(CH, mk(ct), earliest, None) for ct in range(DT)]

            def pump(budget, now=10**9):
                while fillers and budget > 0 and fillers[0][2] <= now:
                    budget -= _pop_one()

            def k_cover_key(t, j):
                # pair 0 produces k-tiles 0..3 individually, chunks after
                if t == 0 and j < 4:
                    return ("ktile", t, j)
                return ("k", t, j // 4)

            # ---------------- pipelined attention windows ----------------
            windows = [(t, c) for t in range(DT) for c in range(NCH)]

            def emit_scores(w, j):
                t, c = windows[w]
                need(k_cover_key(t, j), ("q", t, c))
                sc = psc.tile([P, 1024], dt.float32, tag="sc", name=f"sc{t}{c}{j}")
                nc.tensor.matmul(
                    sc[:, 0:512],
                    kt[0:HD, t, j * P : (j + 1) * P],
                    qt[0:HD, t, c * CH : (c + 1) * CH],
                    start=True,
                    stop=True,
                )
                nc.tensor.matmul(
                    sc[:, 512:1024],
                    kt[HD:P, t, j * P : (j + 1) * P],
                    qt[HD:P, t, c * CH : (c + 1) * CH],
                    start=True,
                    stop=True,
                )
                return sc

            # preamble: K tile j0 (smallest work unblocking the first score
            # matmul), Q(0,c0) in two 256-halves matching the split xt DMA,
            # then the first score tile.
            for group in (k_tile(0, 0), q_half(0, 0, 0), q_half(0, 0, 1)):
                for _, f, _e, key in group:
                    f()
                    if key is not None:
                        done_keys.add(key)
            done_keys.add(("q", 0, 0))
            # pair-0 fillers, ordered by earliest-emission gate (a gated head
            # blocks the whole queue): K tile j is needed by score j, K
            # chunk c by scores 4c.. (xt chunk c DMA), V s-tile st by PV(st)
            # (wv + xt chunk st//4 DMA), Q chunk c by window (0, c).
            fillers.extend(k_tile(0, 1, 0))
            fillers.extend(v_chunk(0, 0, 0))
            fillers.extend(k_tile(0, 2, 1))
            fillers.extend(v_chunk(0, 1, 1))
            fillers.extend(k_tile(0, 3, 2))
            fillers.extend(v_chunk(0, 2, 2))
            fillers.extend(v_chunk(0, 3, 3))
            fillers.extend(k_chunk(0, 1, 4))
            for st in range(4, 8):
                fillers.extend(v_chunk(0, st, 4 + (st - 4)))
            fillers.extend(k_chunk(0, 2, 8))
            for st in range(8, 12):
                fillers.extend(v_chunk(0, st, 8 + (st - 8)))
            fillers.extend(k_chunk(0, 3, 12))
            for st in range(12, ST):
                fillers.extend(v_chunk(0, st, 12 + (st - 12)))
            fillers.extend(q_chunk(0, 1, 13))
            fillers.extend(q_chunk(0, 2, 14))
            fillers.extend(q_chunk(0, 3, 16))

            sc_next = emit_scores(0, 0)

            for w, (t, c) in enumerate(windows):
                # window prologue: queue fillers
                if c == 0 and t + 1 < DT:
                    # next pair's projections (deadlines a full pair away;
                    # pair-1's gate also covers the rest-weight DMAs landing)
                    gate = w * ST + (20 if t == 0 else 0)
                    for cc in range(NCH):
                        fillers.extend(k_chunk(t + 1, cc, gate))
                    for st in range(ST):
                        fillers.extend(v_chunk(t + 1, st, gate))
                    for cc in range(NCH):
                        fillers.extend(q_chunk(t + 1, cc, gate))
                if t == DT - 1 and c >= 1:
                    # SC^T for q-range c-1 is complete across all pairs; gate
                    # two iterations in so its DVE evict lands first
                    for t2 in range(ET):
                        fillers.extend(out_chunk(t2, c - 1, w * ST + 2))
                qh0_pending = t == DT - 1 and c == NCH - 1

                pvh = [
                    ppv.tile(
                        [P, 4 * (HD + 1)], dt.float32, tag="pv", name=f"pv{t}{c}{i}"
                    )
                    for i in range(2)
                ]
                for j in range(ST):
                    sc = sc_next
                    p = ppool.tile([P, 1024], dt.bfloat16, tag="p")
                    nc.scalar.activation(
                        p[:], sc[:], mybir.ActivationFunctionType.Exp, scale=SCALE
                    )
                    # fillers first so produced kt/qt precede dependent scores
                    # in the in-order PE queue (avoids head-of-line blocking).
                    # Budget: generous while draining the pair-0 backlog
                    # (Act is DMA-gated then anyway), just under Act pace in
                    # steady state so the PE never out-runs into Act stalls.
                    pump(1600 if t == 0 else 1000, w * ST + j)
                    if j < ST - 1:
                        sc_next = emit_scores(w, j + 1)
                    elif w + 1 < len(windows):
                        sc_next = emit_scores(w + 1, 0)
                    need(("v", t, j))
                    for h_i in range(2):
                        for qs in range(4):
                            nc.tensor.matmul(
                                pvh[h_i][:, qs * (HD + 1) : (qs + 1) * (HD + 1)],
                                p[:, h_i * 512 + qs * P : h_i * 512 + (qs + 1) * P],
                                va[:, j, 2 * t + h_i, :],
                                start=(j == 0 and qs == 0),
                                stop=(j == ST - 1 and qs == 3),
                            )
                # window epilogue: normalize + transpose
                scns = [[None] * 4 for _ in range(2)]
                for h_i in range(2):
                    rr = rrpool.tile([P, 4], dt.float32, tag="rr", name=f"rr{t}{c}{h_i}")
                    nc.vector.reciprocal(
                        rr[:], pvh[h_i][:, HD : 4 * (HD + 1) : HD + 1]
                    )
                    for qs in range(4):
                        scn = scnpool.tile([P, HD], dt.bfloat16, tag="scn")
                        nc.vector.tensor_scalar_mul(
                            scn[:],
                            pvh[h_i][:, qs * (HD + 1) : qs * (HD + 1) + HD],
                            rr[:, qs : qs + 1],
                        )
                        scns[h_i][qs] = scn
                sct_ps = ppv.tile([P, CH], dt.bfloat16, tag="pv", name=f"tp{t}{c}")
                for h_i in range(2):
                    for qs in range(4):
                        nc.tensor.transpose(
                            sct_ps[h_i * HD : (h_i + 1) * HD, qs * P : (qs + 1) * P],
                            scns[h_i][qs][:],
                            ident[:],
                        )
                nc.vector.tensor_copy(sct[:, t, c * CH : (c + 1) * CH], sct_ps[:])
                if qh0_pending:
                    nc.sync.dma_start(
                        out_d[:, 0:1024].rearrange("(t p) q -> p t q", p=P),
                        ev[:, :, 0:1024],
                    )

            # ---------------- tail: leftover qr2, then final q-range ------
            pump(10**9)
            nc.sync.dma_start(
                out_d[:, 1024:1536].rearrange("(t p) q -> p t q", p=P),
                ev[:, :, 1024:1536],
            )
            for t2 in range(ET):
                for _, f, _e, _k in out_chunk(t2, 3):
                    f()
                # ship each e'-tile as soon as it is evicted so the last DMA
                # only covers one tile
                nc.sync.dma_start(
                    out_d[t2 * P : (t2 + 1) * P, 1536:2048],
                    ev[:, t2, 1536:2048],
                )

    nc.compile()
    return nc


def _prep_inputs(x, W_qkv, b_qkv, W_out, b_out):
    """Host-side sharding + layout prep. Returns per-core input maps."""
    w = W_qkv.reshape(E, H, 3, HD)
    wq_f = np.ascontiguousarray(w[:, :, 0, :].reshape(E, E)).astype(_BF16)
    wk_f = np.ascontiguousarray(w[:, :, 1, :].reshape(E, E)).astype(_BF16)
    wv_f = np.ascontiguousarray(w[:, :, 2, :].reshape(E, E)).astype(_BF16)
    b3 = b_qkv.reshape(H, 3, HD)
    bq_f = np.ascontiguousarray(b3[:, 0, :].reshape(E)).astype(np.float32)

    in_maps = []
    xts = [np.ascontiguousarray(x[b].T).astype(_BF16) for b in range(B)]
    halves = []
    for hh in range(2):
        cols = slice(hh * 512, (hh + 1) * 512)
        def pair_major(wf):
            # [E, 512] -> [P, DT, ET, P]: partition = e-within-tile, then
            # (pair, e-tile, col-within-pair) so per-pair slices are contiguous
            return np.ascontiguousarray(
                wf[:, cols].reshape(ET, P, DT, P).transpose(1, 2, 0, 3)
            )

        wq = pair_major(wq_f)
        wk = pair_major(wk_f)
        wv = pair_major(wv_f)
        wo = np.ascontiguousarray(
            W_out[hh * 512 : (hh + 1) * 512, :].reshape(DT, P, E).transpose(1, 0, 2)
        ).astype(_BF16)
        bq = np.ascontiguousarray(bq_f[cols].reshape(DT, P).T)
        halves.append({"wq": wq, "wk": wk, "wv": wv, "wo": wo, "bq": bq})
    for core in range(N_CORES):
        b, hh = core // 2, core % 2
        m = {"xt": xts[b]}
        m.update(halves[hh])
        in_maps.append(m)
    return in_maps


def run_raw(x, W_qkv, b_qkv, W_out, b_out, trace=False, **kw):
    """Run on hardware; returns (full_output [B,S,E] f32, BassKernelResults)."""
    global _cached
    from concourse.bass_utils import run_bass_kernel_spmd

    if _cached is None:
        _cached = _build()
    nc = _cached
    in_maps = _prep_inputs(x, W_qkv, b_qkv, W_out, b_out)
    res = run_bass_kernel_spmd(
        nc, in_maps, core_ids=list(range(N_CORES)), trace=trace, **kw
    )
    # host: sum row-split partials in fp32, transpose, add fused bias row
    bv_f = np.asarray(b_qkv, dtype=np.float64).reshape(H, 3, HD)[:, 2, :].reshape(E)
    bias_row = (
        np.asarray(b_out, dtype=np.float64) + bv_f @ np.asarray(W_out, dtype=np.float64)
    ).astype(np.float32)
    out = np.empty((B, S, E), dtype=np.float32)
    for b in range(B):
        o0 = np.asarray(res.results[2 * b]["out"]).astype(np.float32)
        o1 = np.asarray(res.results[2 * b + 1]["out"]).astype(np.float32)
        out[b] = (o0 + o1).T + bias_row
    return out, res


def kernel(x, W_qkv, b_qkv, W_out, b_out):
    out, _ = run_raw(x, W_qkv, b_qkv, W_out, b_out)
    return out


# revision 11
# speedup vs baseline: 1.6206x; 1.0327x over previous
"""Multi-head attention (B=4, S=2048, E=1024, H=16) on 8 TRN2 NeuronCores.

Sharding: batch x head-half tensor parallel -- core c handles batch c//2,
heads (c%2)*8 .. (c%2)*8+8, for ALL 2048 query rows.  QKV projections are
column-split by head (no redundant work); the output projection is
row-split: each core produces a partial out^T [E, S] (bf16) and the HOST
sums the two partials per batch in fp32 (plus the fused bias row
b_out + b_v @ W_out).

Key layout/efficiency choices (cost model: matmul time = streamed columns):
  - x arrives as x^T [E, S]; Q^T/K^T [d, s] via matmul(lhsT=W_slice, rhs=x^T).
  - scores transposed: S^T[k, q] = matmul(lhsT=K^T tile, rhs=Q^T), streamed
    over q; exp on ScalarE from PSUM (the attention-phase pacer).
  - PV computed NON-transposed: PV[q, d] = matmul(lhsT=p[k, q-tile],
    rhs=V_aug[k, 65]) -- streams only 65 columns per (q-tile, k-tile), half
    the PE work of the transposed form.  The ones column of V_aug yields the
    softmax denominators in column 64; normalization is a per-partition
    tensor_scalar multiply with the reciprocal of the strided denominators.
  - normalized SC[q, d] tiles are transposed back to SC^T via PE transpose
    (identity matmul) for the output projection, which streams q.
  - K bias is dropped entirely (softmax-invariant); V bias and out bias are
    folded into a host-side bias row (b_out + b_v @ W_out) added after the
    partial sum; only the Q bias is applied in-kernel.
  - score matmuls are software-pipelined one j-step ahead (across window
    boundaries too) so the Activation engine never waits; "filler"
    projection / output matmuls are column-budget pumped into each j
    iteration, deadline-ordered against the progressive x^T DMA arrival,
    so the PE never idles while Activation catches up.

Compute bf16, fp32 PSUM accumulation.
"""

import sys

if "/opt/trn_rl_repo" not in sys.path:
    sys.path.insert(0, "/opt/trn_rl_repo")

import numpy as np
import ml_dtypes

B, S, E, H = 4, 2048, 1024, 16
HD = E // H  # 64
N_CORES = 8
P = 128
ET = E // P          # 8 e-tiles (contraction tiles for projections)
ST = S // P          # 16 s-tiles (k-tiles)
HPC = 8              # heads per core
DT = HPC * HD // P   # 4 d-tiles per core (2 heads per tile)
NCH = 4              # q-chunks of 512
CH = 512

_BF16 = ml_dtypes.bfloat16

_cached = None

SCALE = 1.0 / float(np.sqrt(HD))


def _build():
    import concourse.bass as bass
    import concourse.tile as tile
    import concourse.mybir as mybir
    from concourse import bacc
    from concourse.masks import make_identity
    from collections import deque

    dt = mybir.dt
    nc = bacc.Bacc("TRN2", target_bir_lowering=False, debug=False)

    xt_d = nc.dram_tensor("xt", [E, S], dt.bfloat16, kind="ExternalInput").ap()
    wq_d = nc.dram_tensor("wq", [P, DT, ET, P], dt.bfloat16, kind="ExternalInput").ap()
    wk_d = nc.dram_tensor("wk", [P, DT, ET, P], dt.bfloat16, kind="ExternalInput").ap()
    wv_d = nc.dram_tensor("wv", [P, DT, ET, P], dt.bfloat16, kind="ExternalInput").ap()
    wo_d = nc.dram_tensor("wo", [P, DT, E], dt.bfloat16, kind="ExternalInput").ap()
    bq_d = nc.dram_tensor("bq", [P, DT], dt.float32, kind="ExternalInput").ap()
    out_d = nc.dram_tensor("out", [E, S], dt.bfloat16, kind="ExternalOutput").ap()

    with tile.TileContext(nc) as tc:
        with (
            tc.tile_pool(name="const", bufs=1) as cpool,
            tc.tile_pool(name="acts", bufs=1) as apool,
            tc.tile_pool(name="p", bufs=6) as ppool,
            tc.tile_pool(name="scn", bufs=10) as scnpool,
            tc.tile_pool(name="rr", bufs=4) as rrpool,
            tc.tile_pool(name="psc", bufs=2, space="PSUM") as psc,
            tc.tile_pool(name="ppv", bufs=2, space="PSUM") as ppv,
            tc.tile_pool(name="pproj", bufs=2, space="PSUM") as pproj,
        ):
            # ---------------- constants / DMAs ----------------
            xt = cpool.tile([P, ET, S], dt.bfloat16)
            wq = cpool.tile([P, DT, ET, P], dt.bfloat16)
            wk = cpool.tile([P, DT, ET, P], dt.bfloat16)
            wv = cpool.tile([P, DT, ET, P], dt.bfloat16)
            wo = cpool.tile([P, DT, E], dt.bfloat16)
            bq = cpool.tile([P, DT], dt.float32)
            ident = cpool.tile([P, P], dt.bfloat16)

            # DMA schedule: the startup critical path is this serial chain,
            # so pair-0's weight slices (contiguous in the pair-major DRAM
            # layout, ~0.7us each) and the first half of xt chunk 0 come
            # first; the rest follows in deadline order; wo is needed last.
            def xt_cols_dma(lo, hi):
                nc.sync.dma_start(
                    xt[:, :, lo:hi],
                    xt_d[:, lo:hi].rearrange("(eo p) s -> p eo s", p=P),
                )

            nc.sync.dma_start(wq[:, 0], wq_d[:, 0])
            xt_cols_dma(0, 256)
            nc.sync.dma_start(wk[:, 0], wk_d[:, 0])
            xt_cols_dma(256, 512)
            nc.sync.dma_start(wv[:, 0], wv_d[:, 0])
            nc.sync.dma_start(bq[:], bq_d)
            for m in range(2, 8):
                xt_cols_dma(m * 256, (m + 1) * 256)
            nc.sync.dma_start(wq[:, 1:], wq_d[:, 1:])
            nc.sync.dma_start(wk[:, 1:], wk_d[:, 1:])
            nc.sync.dma_start(wv[:, 1:], wv_d[:, 1:])
            nc.sync.dma_start(wo[:], wo_d)
            make_identity(nc, ident[:])

            qt = apool.tile([P, DT, S], dt.bfloat16)   # Q^T + bq
            kt = apool.tile([P, DT, S], dt.bfloat16)   # K^T (no bias)
            va = apool.tile([P, ST, HPC, HD + 1], dt.bfloat16)  # V | ones col
            sct = apool.tile([P, DT, S], dt.bfloat16)  # normalized SC^T
            ev = apool.tile([P, ET, S], dt.bfloat16)   # staged out^T partial

            nc.gpsimd.memset(va[:, :, :, HD : HD + 1], 1.0)

            # ---------------- filler machinery ----------------
            # Fillers are (cols, callable, earliest) triples, each emitting
            # ONE PE matmul (chunk evictions are emitted inline by the
            # chunk's last matmul; they run on DVE and cost no PE time).
            # pump(budget, now) pops fillers until the column budget is
            # spent, but never emits a filler before its `earliest`
            # window*16+j position -- this keeps DMA-gated matmuls from
            # entering the in-order PE queue early and head-of-line blocking
            # the attention stream behind them.
            fillers = deque()
            done_keys = set()

            def _pop_one():
                cols, fn, _e, key = fillers.popleft()
                fn()
                if key is not None:
                    done_keys.add(key)
                return cols

            def need(*keys):
                # correctness guard: force-emit queued fillers (in order,
                # ignoring gates/budgets) until every producer key a consumer
                # is about to read has been emitted.  Emission order is what
                # the tile framework builds dependencies from.
                for key in keys:
                    while key not in done_keys:
                        assert fillers, f"missing producer for {key}"
                        _pop_one()

            def q_chunk(t, c, earliest=0):
                state = {}
                key = ("q", t, c)

                def mk(e):
                    def emit():
                        if e == 0:
                            state["ps"] = pproj.tile(
                                [P, CH], dt.float32, tag="proj", name=f"q{t}{c}"
                            )
                        nc.tensor.matmul(
                            state["ps"][:],
                            wq[:, t, e, :],
                            xt[:, e, c * CH : (c + 1) * CH],
                            start=(e == 0),
                            stop=(e == ET - 1),
                        )
                        if e == ET - 1:
                            nc.vector.tensor_scalar_add(
                                qt[:, t, c * CH : (c + 1) * CH],
                                state["ps"][:],
                                bq[:, t : t + 1],
                            )

                    return emit

                return [(CH, mk(e), earliest, key if e == ET - 1 else None) for e in range(ET)]

            def k_chunk(t, c, earliest=0):
                state = {}
                key = ("k", t, c)

                def mk(e):
                    def emit():
                        if e == 0:
                            state["ps"] = pproj.tile(
                                [P, CH], dt.float32, tag="proj", name=f"k{t}{c}"
                            )
                        nc.tensor.matmul(
                            state["ps"][:],
                            wk[:, t, e, :],
                            xt[:, e, c * CH : (c + 1) * CH],
                            start=(e == 0),
                            stop=(e == ET - 1),
                        )
                        if e == ET - 1:
                            nc.vector.tensor_copy(
                                kt[:, t, c * CH : (c + 1) * CH], state["ps"][:]
                            )

                    return emit

                return [(CH, mk(e), earliest, key if e == ET - 1 else None) for e in range(ET)]

            def k_tile(t, j, earliest=0):
                # fine-grained K chunk: a single 128-wide k-tile
                state = {}
                key = ("ktile", t, j)

                def mk(e):
                    def emit():
                        if e == 0:
                            state["ps"] = pproj.tile(
                                [P, P], dt.float32, tag="proj", name=f"kt{t}_{j}"
                            )
                        nc.tensor.matmul(
                            state["ps"][:],
                            wk[:, t, e, :],
                            xt[:, e, j * P : (j + 1) * P],
                            start=(e == 0),
                            stop=(e == ET - 1),
                        )
                        if e == ET - 1:
                            nc.vector.tensor_copy(
                                kt[:, t, j * P : (j + 1) * P], state["ps"][:]
                            )

                    return emit

                return [(P, mk(e), earliest, key if e == ET - 1 else None) for e in range(ET)]

            def q_half(t, c, half):
                # 256-wide Q chunk (startup granularity, xt arrives in halves)
                lo = c * CH + half * 256
                state = {}
                key = ("qhalf", t, c, half)

                def mk(e):
                    def emit():
                        if e == 0:
                            state["ps"] = pproj.tile(
                                [P, 256], dt.float32, tag="proj", name=f"qh{t}{c}{half}"
                            )
                        nc.tensor.matmul(
                            state["ps"][:],
                            wq[:, t, e, :],
                            xt[:, e, lo : lo + 256],
                            start=(e == 0),
                            stop=(e == ET - 1),
                        )
                        if e == ET - 1:
                            nc.vector.tensor_scalar_add(
                                qt[:, t, lo : lo + 256],
                                state["ps"][:],
                                bq[:, t : t + 1],
                            )

                    return emit

                return [(256, mk(e), 0, key if e == ET - 1 else None) for e in range(ET)]

            def v_chunk(t, st, earliest=0):
                state = {}
                key = ("v", t, st)

                def mk(e):
                    def emit():
                        if e == 0:
                            state["ps"] = pproj.tile(
                                [P, 2 * HD], dt.float32, tag="proj", name=f"v{t}{st}"
                            )
                        nc.tensor.matmul(
                            state["ps"][:],
                            xt[:, e, st * P : (st + 1) * P],
                            wv[:, t, e, :],
                            start=(e == 0),
                            stop=(e == ET - 1),
                        )
                        if e == ET - 1:
                            nc.vector.tensor_copy(
                                va[:, st, 2 * t : 2 * t + 2, 0:HD],
                                state["ps"].rearrange("p (h d) -> p h d", d=HD),
                            )

                    return emit

                return [(2 * HD, mk(e), earliest, key if e == ET - 1 else None) for e in range(ET)]

            def out_chunk(t2, qr, earliest=0):
                state = {}

                def mk(ct):
                    def emit():
                        if ct == 0:
                            state["ps"] = pproj.tile(
                                [P, CH], dt.float32, tag="proj", name=f"o{t2}{qr}"
                            )
                        nc.tensor.matmul(
                            state["ps"][:],
                            wo[:, ct, t2 * P : (t2 + 1) * P],
                            sct[:, ct, qr * CH : (qr + 1) * CH],
                            start=(ct == 0),
                            stop=(ct == DT - 1),
                        )
                        if ct == DT - 1:
                            nc.vector.tensor_copy(
                                ev[:, t2, qr * CH : (qr + 1) * CH], state["ps"][:]
                            )

                    return emit

                return [(CH, mk(ct), earliest, None) for ct in range(DT)]

            def pump(budget, now=10**9):
                while fillers and budget > 0 and fillers[0][2] <= now:
                    budget -= _pop_one()

            def k_cover_key(t, j):
                # pair 0 produces all k-tiles individually, chunks for others
                if t == 0:
                    return ("ktile", t, j)
                return ("k", t, j // 4)

            # ---------------- pipelined attention windows ----------------
            windows = [(t, c) for t in range(DT) for c in range(NCH)]

            def emit_scores(w, j):
                t, c = windows[w]
                need(k_cover_key(t, j), ("q", t, c))
                sc = psc.tile([P, 1024], dt.float32, tag="sc", name=f"sc{t}{c}{j}")
                nc.tensor.matmul(
                    sc[:, 0:512],
                    kt[0:HD, t, j * P : (j + 1) * P],
                    qt[0:HD, t, c * CH : (c + 1) * CH],
                    start=True,
                    stop=True,
                )
                nc.tensor.matmul(
                    sc[:, 512:1024],
                    kt[HD:P, t, j * P : (j + 1) * P],
                    qt[HD:P, t, c * CH : (c + 1) * CH],
                    start=True,
                    stop=True,
                )
                return sc

            # preamble: K tile j0 (smallest work unblocking the first score
            # matmul), Q(0,c0) in two 256-halves matching the split xt DMA,
            # then the first score tile.
            for group in (k_tile(0, 0), q_half(0, 0, 0), q_half(0, 0, 1)):
                for _, f, _e, key in group:
                    f()
                    if key is not None:
                        done_keys.add(key)
            done_keys.add(("q", 0, 0))
            # pair-0 fillers, ordered by earliest-emission gate (a gated head
            # blocks the whole queue): K tile j is needed by score j, K
            # chunk c by scores 4c.. (xt chunk c DMA), V s-tile st by PV(st)
            # (wv + xt chunk st//4 DMA), Q chunk c by window (0, c).
            for j in range(1, ST):
                gate = max(0, j - 3)
                fillers.extend(k_tile(0, j, gate))
                fillers.extend(v_chunk(0, j - 1, gate))
                if j == 7:
                    fillers.extend(q_chunk(0, 1, 5))
                elif j == 11:
                    fillers.extend(q_chunk(0, 2, 9))
            fillers.extend(v_chunk(0, ST - 1, 13))
            fillers.extend(q_chunk(0, 3, 13))

            sc_next = emit_scores(0, 0)

            for w, (t, c) in enumerate(windows):
                # window prologue: queue fillers
                if c == 0 and t + 1 < DT:
                    # next pair's projections, gated just-in-time (one window
                    # of margin before each chunk's first consumer) so the
                    # filler stream stays flat instead of bursting into the
                    # early windows and stalling the Activation engine.
                    # Pair-1 additionally waits for the rest-weight DMAs.
                    base = (t + 1) * 4 * ST
                    floor = w * ST + (20 if t == 0 else 0)

                    def g(rel):
                        return max(base + rel, floor)

                    fillers.extend(q_chunk(t + 1, 0, g(-16)))
                    fillers.extend(k_chunk(t + 1, 0, g(-16)))
                    for st in range(4):
                        fillers.extend(v_chunk(t + 1, st, g(st - 16)))
                    fillers.extend(k_chunk(t + 1, 1, g(-12)))
                    for st in range(4, 8):
                        fillers.extend(v_chunk(t + 1, st, g(st - 16)))
                    fillers.extend(k_chunk(t + 1, 2, g(-8)))
                    for st in range(8, 12):
                        fillers.extend(v_chunk(t + 1, st, g(st - 16)))
                    fillers.extend(k_chunk(t + 1, 3, g(-4)))
                    for st in range(12, ST):
                        fillers.extend(v_chunk(t + 1, st, g(st - 16)))
                    fillers.extend(q_chunk(t + 1, 1, g(0)))
                    fillers.extend(q_chunk(t + 1, 2, g(16)))
                    fillers.extend(q_chunk(t + 1, 3, g(32)))
                if t == DT - 1 and c >= 1:
                    # SC^T for q-range c-1 is complete across all pairs; gate
                    # two iterations in so its DVE evict lands first
                    for t2 in range(ET):
                        fillers.extend(out_chunk(t2, c - 1, w * ST + 2))
                qh0_pending = t == DT - 1 and c == NCH - 1

                pvh = [
                    ppv.tile(
                        [P, 4 * (HD + 1)], dt.float32, tag="pv", name=f"pv{t}{c}{i}"
                    )
                    for i in range(2)
                ]
                for j in range(ST):
                    sc = sc_next
                    p = ppool.tile([P, 1024], dt.bfloat16, tag="p")
                    nc.scalar.activation(
                        p[:], sc[:], mybir.ActivationFunctionType.Exp, scale=SCALE
                    )
                    # fillers first so produced kt/qt precede dependent scores
                    # in the in-order PE queue (avoids head-of-line blocking).
                    # Budget: generous while draining the pair-0 backlog
                    # (Act is DMA-gated then anyway), just under Act pace in
                    # steady state so the PE never out-runs into Act stalls.
                    if w < 2:
                        budget = 1500
                    elif t == DT - 1 and c == NCH - 1:
                        budget = 1400
                    else:
                        budget = 1250
                    pump(budget, w * ST + j)
                    if j < ST - 1:
                        sc_next = emit_scores(w, j + 1)
                    elif w + 1 < len(windows):
                        sc_next = emit_scores(w + 1, 0)
                    need(("v", t, j))
                    for h_i in range(2):
                        for qs in range(4):
                            nc.tensor.matmul(
                                pvh[h_i][:, qs * (HD + 1) : (qs + 1) * (HD + 1)],
                                p[:, h_i * 512 + qs * P : h_i * 512 + (qs + 1) * P],
                                va[:, j, 2 * t + h_i, :],
                                start=(j == 0 and qs == 0),
                                stop=(j == ST - 1 and qs == 3),
                            )
                # window epilogue: normalize + transpose
                scns = [[None] * 4 for _ in range(2)]
                for h_i in range(2):
                    rr = rrpool.tile([P, 4], dt.float32, tag="rr", name=f"rr{t}{c}{h_i}")
                    nc.vector.reciprocal(
                        rr[:], pvh[h_i][:, HD : 4 * (HD + 1) : HD + 1]
                    )
                    for qs in range(4):
                        scn = scnpool.tile([P, HD], dt.bfloat16, tag="scn")
                        nc.vector.tensor_scalar_mul(
                            scn[:],
                            pvh[h_i][:, qs * (HD + 1) : qs * (HD + 1) + HD],
                            rr[:, qs : qs + 1],
                        )
                        scns[h_i][qs] = scn
                sct_ps = ppv.tile([P, CH], dt.bfloat16, tag="pv", name=f"tp{t}{c}")
                for h_i in range(2):
                    for qs in range(4):
                        nc.tensor.transpose(
                            sct_ps[h_i * HD : (h_i + 1) * HD, qs * P : (qs + 1) * P],
                            scns[h_i][qs][:],
                            ident[:],
                        )
                nc.vector.tensor_copy(sct[:, t, c * CH : (c + 1) * CH], sct_ps[:])
                if qh0_pending:
                    nc.sync.dma_start(
                        out_d[:, 0:1024].rearrange("(t p) q -> p t q", p=P),
                        ev[:, :, 0:1024],
                    )

            # ---------------- tail: leftover qr2, then final q-range ------
            pump(10**9)
            nc.sync.dma_start(
                out_d[:, 1024:1536].rearrange("(t p) q -> p t q", p=P),
                ev[:, :, 1024:1536],
            )
            for t2 in range(ET):
                for _, f, _e, _k in out_chunk(t2, 3):
                    f()
                # ship each e'-tile as soon as it is evicted so the last DMA
                # only covers one tile
                nc.sync.dma_start(
                    out_d[t2 * P : (t2 + 1) * P, 1536:2048],
                    ev[:, t2, 1536:2048],
                )

    nc.compile()
    return nc


def _prep_inputs(x, W_qkv, b_qkv, W_out, b_out):
    """Host-side sharding + layout prep. Returns per-core input maps."""
    w = W_qkv.reshape(E, H, 3, HD)
    wq_f = np.ascontiguousarray(w[:, :, 0, :].reshape(E, E)).astype(_BF16)
    wk_f = np.ascontiguousarray(w[:, :, 1, :].reshape(E, E)).astype(_BF16)
    wv_f = np.ascontiguousarray(w[:, :, 2, :].reshape(E, E)).astype(_BF16)
    b3 = b_qkv.reshape(H, 3, HD)
    bq_f = np.ascontiguousarray(b3[:, 0, :].reshape(E)).astype(np.float32)

    in_maps = []
    xts = [np.ascontiguousarray(x[b].T).astype(_BF16) for b in range(B)]
    halves = []
    for hh in range(2):
        cols = slice(hh * 512, (hh + 1) * 512)
        def pair_major(wf):
            # [E, 512] -> [P, DT, ET, P]: partition = e-within-tile, then
            # (pair, e-tile, col-within-pair) so per-pair slices are contiguous
            return np.ascontiguousarray(
                wf[:, cols].reshape(ET, P, DT, P).transpose(1, 2, 0, 3)
            )

        wq = pair_major(wq_f)
        wk = pair_major(wk_f)
        wv = pair_major(wv_f)
        wo = np.ascontiguousarray(
            W_out[hh * 512 : (hh + 1) * 512, :].reshape(DT, P, E).transpose(1, 0, 2)
        ).astype(_BF16)
        bq = np.ascontiguousarray(bq_f[cols].reshape(DT, P).T)
        halves.append({"wq": wq, "wk": wk, "wv": wv, "wo": wo, "bq": bq})
    for core in range(N_CORES):
        b, hh = core // 2, core % 2
        m = {"xt": xts[b]}
        m.update(halves[hh])
        in_maps.append(m)
    return in_maps


def run_raw(x, W_qkv, b_qkv, W_out, b_out, trace=False, **kw):
    """Run on hardware; returns (full_output [B,S,E] f32, BassKernelResults)."""
    global _cached
    from concourse.bass_utils import run_bass_kernel_spmd

    if _cached is None:
        _cached = _build()
    nc = _cached
    in_maps = _prep_inputs(x, W_qkv, b_qkv, W_out, b_out)
    res = run_bass_kernel_spmd(
        nc, in_maps, core_ids=list(range(N_CORES)), trace=trace, **kw
    )
    # host: sum row-split partials in fp32, transpose, add fused bias row
    bv_f = np.asarray(b_qkv, dtype=np.float64).reshape(H, 3, HD)[:, 2, :].reshape(E)
    bias_row = (
        np.asarray(b_out, dtype=np.float64) + bv_f @ np.asarray(W_out, dtype=np.float64)
    ).astype(np.float32)
    out = np.empty((B, S, E), dtype=np.float32)
    for b in range(B):
        o0 = np.asarray(res.results[2 * b]["out"]).astype(np.float32)
        o1 = np.asarray(res.results[2 * b + 1]["out"]).astype(np.float32)
        out[b] = (o0 + o1).T + bias_row
    return out, res


def kernel(x, W_qkv, b_qkv, W_out, b_out):
    out, _ = run_raw(x, W_qkv, b_qkv, W_out, b_out)
    return out


# revision 12
# speedup vs baseline: 1.6432x; 1.0140x over previous
"""Multi-head attention (B=4, S=2048, E=1024, H=16) on 8 TRN2 NeuronCores.

Sharding: batch x head-half tensor parallel -- core c handles batch c//2,
heads (c%2)*8 .. (c%2)*8+8, for ALL 2048 query rows.  QKV projections are
column-split by head (no redundant work); the output projection is
row-split: each core produces a partial out^T [E, S] (bf16) and the HOST
sums the two partials per batch in fp32 (plus the fused bias row
b_out + b_v @ W_out).

Key layout/efficiency choices (cost model: matmul time = streamed columns):
  - x arrives as x^T [E, S]; Q^T/K^T [d, s] via matmul(lhsT=W_slice, rhs=x^T).
  - scores transposed: S^T[k, q] = matmul(lhsT=K^T tile, rhs=Q^T), streamed
    over q; exp on ScalarE from PSUM (the attention-phase pacer).
  - PV computed NON-transposed: PV[q, d] = matmul(lhsT=p[k, q-tile],
    rhs=V_aug[k, 65]) -- streams only 65 columns per (q-tile, k-tile), half
    the PE work of the transposed form.  The ones column of V_aug yields the
    softmax denominators in column 64; normalization is a per-partition
    tensor_scalar multiply with the reciprocal of the strided denominators.
  - normalized SC[q, d] tiles are transposed back to SC^T via PE transpose
    (identity matmul) for the output projection, which streams q.
  - K bias is dropped entirely (softmax-invariant); V bias and out bias are
    folded into a host-side bias row (b_out + b_v @ W_out) added after the
    partial sum; only the Q bias is applied in-kernel.
  - score matmuls are software-pipelined one j-step ahead (across window
    boundaries too) so the Activation engine never waits; "filler"
    projection / output matmuls are column-budget pumped into each j
    iteration, deadline-ordered against the progressive x^T DMA arrival,
    so the PE never idles while Activation catches up.

Compute bf16, fp32 PSUM accumulation.
"""

import sys

if "/opt/trn_rl_repo" not in sys.path:
    sys.path.insert(0, "/opt/trn_rl_repo")

import numpy as np
import ml_dtypes

B, S, E, H = 4, 2048, 1024, 16
HD = E // H  # 64
N_CORES = 8
P = 128
ET = E // P          # 8 e-tiles (contraction tiles for projections)
ST = S // P          # 16 s-tiles (k-tiles)
HPC = 8              # heads per core
DT = HPC * HD // P   # 4 d-tiles per core (2 heads per tile)
NCH = 4              # q-chunks of 512
CH = 512

_BF16 = ml_dtypes.bfloat16

_cached = None

SCALE = 1.0 / float(np.sqrt(HD))


def _build():
    import concourse.bass as bass
    import concourse.tile as tile
    import concourse.mybir as mybir
    from concourse import bacc
    from concourse.masks import make_identity
    from collections import deque

    dt = mybir.dt
    nc = bacc.Bacc("TRN2", target_bir_lowering=False, debug=False)

    xt_d = nc.dram_tensor("xt", [E, S], dt.bfloat16, kind="ExternalInput").ap()
    wq_d = nc.dram_tensor("wq", [P, DT, ET, P], dt.bfloat16, kind="ExternalInput").ap()
    wk_d = nc.dram_tensor("wk", [P, DT, ET, P], dt.bfloat16, kind="ExternalInput").ap()
    wv_d = nc.dram_tensor("wv", [P, DT, ET, P], dt.bfloat16, kind="ExternalInput").ap()
    wo_d = nc.dram_tensor("wo", [P, DT, E], dt.bfloat16, kind="ExternalInput").ap()
    bq_d = nc.dram_tensor("bq", [P, DT], dt.float32, kind="ExternalInput").ap()
    out_d = nc.dram_tensor("out", [E, S], dt.bfloat16, kind="ExternalOutput").ap()

    with tile.TileContext(nc) as tc:
        with (
            tc.tile_pool(name="const", bufs=1) as cpool,
            tc.tile_pool(name="acts", bufs=1) as apool,
            tc.tile_pool(name="p", bufs=6) as ppool,
            tc.tile_pool(name="scn", bufs=10) as scnpool,
            tc.tile_pool(name="rr", bufs=4) as rrpool,
            tc.tile_pool(name="psc", bufs=2, space="PSUM") as psc,
            tc.tile_pool(name="ppv", bufs=2, space="PSUM") as ppv,
            tc.tile_pool(name="pproj", bufs=2, space="PSUM") as pproj,
        ):
            # ---------------- constants / DMAs ----------------
            xt = cpool.tile([P, ET, S], dt.bfloat16)
            wq = cpool.tile([P, DT, ET, P], dt.bfloat16)
            wk = cpool.tile([P, DT, ET, P], dt.bfloat16)
            wv = cpool.tile([P, DT, ET, P], dt.bfloat16)
            wo = cpool.tile([P, DT, E], dt.bfloat16)
            bq = cpool.tile([P, DT], dt.float32)
            ident = cpool.tile([P, P], dt.bfloat16)

            # DMA schedule: the startup critical path is this serial chain,
            # so pair-0's weight slices (contiguous in the pair-major DRAM
            # layout, ~0.7us each) and the first half of xt chunk 0 come
            # first; the rest follows in deadline order; wo is needed last.
            def xt_cols_dma(lo, hi):
                nc.sync.dma_start(
                    xt[:, :, lo:hi],
                    xt_d[:, lo:hi].rearrange("(eo p) s -> p eo s", p=P),
                )

            nc.sync.dma_start(wq[:, 0], wq_d[:, 0])
            xt_cols_dma(0, 256)
            nc.sync.dma_start(wk[:, 0], wk_d[:, 0])
            xt_cols_dma(256, 512)
            nc.sync.dma_start(bq[:], bq_d)
            nc.sync.dma_start(wv[:, 0], wv_d[:, 0])
            for m in range(2, 8):
                xt_cols_dma(m * 256, (m + 1) * 256)
            nc.sync.dma_start(wq[:, 1:], wq_d[:, 1:])
            nc.sync.dma_start(wk[:, 1:], wk_d[:, 1:])
            nc.sync.dma_start(wv[:, 1:], wv_d[:, 1:])
            nc.sync.dma_start(wo[:], wo_d)
            make_identity(nc, ident[:])

            qt = apool.tile([P, DT, S], dt.bfloat16)   # Q^T + bq
            kt = apool.tile([P, DT, S], dt.bfloat16)   # K^T (no bias)
            va = apool.tile([P, ST, HPC, HD + 1], dt.bfloat16)  # V | ones col
            sct = apool.tile([P, DT, S], dt.bfloat16)  # normalized SC^T
            ev = apool.tile([P, ET, S], dt.bfloat16)   # staged out^T partial

            nc.gpsimd.memset(va[:, :, :, HD : HD + 1], 1.0)

            # ---------------- filler machinery ----------------
            # Fillers are (cols, callable, earliest) triples, each emitting
            # ONE PE matmul (chunk evictions are emitted inline by the
            # chunk's last matmul; they run on DVE and cost no PE time).
            # pump(budget, now) pops fillers until the column budget is
            # spent, but never emits a filler before its `earliest`
            # window*16+j position -- this keeps DMA-gated matmuls from
            # entering the in-order PE queue early and head-of-line blocking
            # the attention stream behind them.
            fillers = deque()
            done_keys = set()

            def _pop_one():
                cols, fn, _e, key = fillers.popleft()
                fn()
                if key is not None:
                    done_keys.add(key)
                return cols

            def need(*keys):
                # correctness guard: force-emit queued fillers (in order,
                # ignoring gates/budgets) until every producer key a consumer
                # is about to read has been emitted.  Emission order is what
                # the tile framework builds dependencies from.
                for key in keys:
                    while key not in done_keys:
                        assert fillers, f"missing producer for {key}"
                        _pop_one()

            def q_chunk(t, c, earliest=0):
                state = {}
                key = ("q", t, c)

                def mk(e):
                    def emit():
                        if e == 0:
                            state["ps"] = pproj.tile(
                                [P, CH], dt.float32, tag="proj", name=f"q{t}{c}"
                            )
                        nc.tensor.matmul(
                            state["ps"][:],
                            wq[:, t, e, :],
                            xt[:, e, c * CH : (c + 1) * CH],
                            start=(e == 0),
                            stop=(e == ET - 1),
                        )
                        if e == ET - 1:
                            nc.vector.tensor_scalar_add(
                                qt[:, t, c * CH : (c + 1) * CH],
                                state["ps"][:],
                                bq[:, t : t + 1],
                            )

                    return emit

                return [(CH, mk(e), earliest, key if e == ET - 1 else None) for e in range(ET)]

            def k_chunk(t, c, earliest=0):
                state = {}
                key = ("k", t, c)

                def mk(e):
                    def emit():
                        if e == 0:
                            state["ps"] = pproj.tile(
                                [P, CH], dt.float32, tag="proj", name=f"k{t}{c}"
                            )
                        nc.tensor.matmul(
                            state["ps"][:],
                            wk[:, t, e, :],
                            xt[:, e, c * CH : (c + 1) * CH],
                            start=(e == 0),
                            stop=(e == ET - 1),
                        )
                        if e == ET - 1:
                            nc.vector.tensor_copy(
                                kt[:, t, c * CH : (c + 1) * CH], state["ps"][:]
                            )

                    return emit

                return [(CH, mk(e), earliest, key if e == ET - 1 else None) for e in range(ET)]

            def k_tile(t, j, earliest=0):
                # fine-grained K chunk: a single 128-wide k-tile
                state = {}
                key = ("ktile", t, j)

                def mk(e):
                    def emit():
                        if e == 0:
                            state["ps"] = pproj.tile(
                                [P, P], dt.float32, tag="proj", name=f"kt{t}_{j}"
                            )
                        nc.tensor.matmul(
                            state["ps"][:],
                            wk[:, t, e, :],
                            xt[:, e, j * P : (j + 1) * P],
                            start=(e == 0),
                            stop=(e == ET - 1),
                        )
                        if e == ET - 1:
                            nc.vector.tensor_copy(
                                kt[:, t, j * P : (j + 1) * P], state["ps"][:]
                            )

                    return emit

                return [(P, mk(e), earliest, key if e == ET - 1 else None) for e in range(ET)]

            def q_half(t, c, half, earliest=0, width=256):
                # narrow Q chunk (startup / de-clumping granularity)
                lo = c * CH + half * width
                state = {}
                key = ("qhalf", t, c, half)

                def mk(e):
                    def emit():
                        if e == 0:
                            state["ps"] = pproj.tile(
                                [P, width], dt.float32, tag="proj", name=f"qh{t}{c}{half}"
                            )
                        nc.tensor.matmul(
                            state["ps"][:],
                            wq[:, t, e, :],
                            xt[:, e, lo : lo + width],
                            start=(e == 0),
                            stop=(e == ET - 1),
                        )
                        if e == ET - 1:
                            nc.vector.tensor_scalar_add(
                                qt[:, t, lo : lo + width],
                                state["ps"][:],
                                bq[:, t : t + 1],
                            )

                    return emit

                return [
                    (width, mk(e), earliest, key if e == ET - 1 else None)
                    for e in range(ET)
                ]

            def v_chunk(t, st, earliest=0):
                state = {}
                key = ("v", t, st)

                def mk(e):
                    def emit():
                        if e == 0:
                            state["ps"] = pproj.tile(
                                [P, 2 * HD], dt.float32, tag="proj", name=f"v{t}{st}"
                            )
                        nc.tensor.matmul(
                            state["ps"][:],
                            xt[:, e, st * P : (st + 1) * P],
                            wv[:, t, e, :],
                            start=(e == 0),
                            stop=(e == ET - 1),
                        )
                        if e == ET - 1:
                            nc.vector.tensor_copy(
                                va[:, st, 2 * t : 2 * t + 2, 0:HD],
                                state["ps"].rearrange("p (h d) -> p h d", d=HD),
                            )

                    return emit

                return [(2 * HD, mk(e), earliest, key if e == ET - 1 else None) for e in range(ET)]

            def out_chunk(t2, qr, earliest=0):
                state = {}

                def mk(ct):
                    def emit():
                        if ct == 0:
                            state["ps"] = pproj.tile(
                                [P, CH], dt.float32, tag="proj", name=f"o{t2}{qr}"
                            )
                        nc.tensor.matmul(
                            state["ps"][:],
                            wo[:, ct, t2 * P : (t2 + 1) * P],
                            sct[:, ct, qr * CH : (qr + 1) * CH],
                            start=(ct == 0),
                            stop=(ct == DT - 1),
                        )
                        if ct == DT - 1:
                            nc.vector.tensor_copy(
                                ev[:, t2, qr * CH : (qr + 1) * CH], state["ps"][:]
                            )

                    return emit

                return [(CH, mk(ct), earliest, None) for ct in range(DT)]

            def pump(budget, now=10**9):
                while fillers and budget > 0 and fillers[0][2] <= now:
                    budget -= _pop_one()

            def k_cover_key(t, j):
                # pair 0 produces all k-tiles individually, chunks for others
                if t == 0:
                    return ("ktile", t, j)
                return ("k", t, j // 4)

            # ---------------- pipelined attention windows ----------------
            windows = [(t, c) for t in range(DT) for c in range(NCH)]

            q_by_halves = {(0, 0), (0, 1), (0, 2), (0, 3)}

            def q_cover(t, c):
                if (t, c) in q_by_halves:
                    return (("qhalf", t, c, 0), ("qhalf", t, c, 1))
                return (("q", t, c),)

            def emit_scores(w, j):
                t, c = windows[w]
                need(k_cover_key(t, j), *q_cover(t, j // 10**9 if False else c))
                sc = psc.tile([P, 1024], dt.float32, tag="sc", name=f"sc{t}{c}{j}")
                nc.tensor.matmul(
                    sc[:, 0:512],
                    kt[0:HD, t, j * P : (j + 1) * P],
                    qt[0:HD, t, c * CH : (c + 1) * CH],
                    start=True,
                    stop=True,
                )
                nc.tensor.matmul(
                    sc[:, 512:1024],
                    kt[HD:P, t, j * P : (j + 1) * P],
                    qt[HD:P, t, c * CH : (c + 1) * CH],
                    start=True,
                    stop=True,
                )
                return sc

            # preamble: K tile j0 (smallest work unblocking the first score
            # matmul), Q(0,c0) in two 256-halves matching the split xt DMA,
            # then the first score tile.
            for group in (q_half(0, 0, 0), k_tile(0, 0), q_half(0, 0, 1)):
                for _, f, _e, key in group:
                    f()
                    if key is not None:
                        done_keys.add(key)
            done_keys.add(("q", 0, 0))
            # pair-0 fillers, ordered by earliest-emission gate (a gated head
            # blocks the whole queue): K tile j is needed by score j, K
            # chunk c by scores 4c.. (xt chunk c DMA), V s-tile st by PV(st)
            # (wv + xt chunk st//4 DMA), Q chunk c by window (0, c).
            for j in range(1, ST):
                gate = max(0, j - 3)
                fillers.extend(k_tile(0, j, gate))
                fillers.extend(v_chunk(0, j - 1, gate))
                if j == 7:
                    fillers.extend(q_half(0, 1, 0, 5))
                elif j == 9:
                    fillers.extend(q_half(0, 1, 1, 7))
            fillers.extend(v_chunk(0, ST - 1, 13))
            fillers.extend(q_half(0, 2, 0, 20))
            fillers.extend(q_half(0, 2, 1, 23))
            fillers.extend(q_half(0, 3, 0, 36))
            fillers.extend(q_half(0, 3, 1, 39))

            sc_next = emit_scores(0, 0)

            for w, (t, c) in enumerate(windows):
                # window prologue: queue fillers
                if c == 0 and t + 1 < DT:
                    # next pair's projections, gated just-in-time (one window
                    # of margin before each chunk's first consumer) so the
                    # filler stream stays flat instead of bursting into the
                    # early windows and stalling the Activation engine.
                    # Pair-1 additionally waits for the rest-weight DMAs.
                    base = (t + 1) * 4 * ST
                    floor = w * ST + (20 if t == 0 else 0)

                    def g(rel):
                        return max(base + rel, floor)

                    fillers.extend(q_chunk(t + 1, 0, g(-16)))
                    fillers.extend(k_chunk(t + 1, 0, g(-16)))
                    for st in range(4):
                        fillers.extend(v_chunk(t + 1, st, g(st - 16)))
                    fillers.extend(k_chunk(t + 1, 1, g(-12)))
                    for st in range(4, 8):
                        fillers.extend(v_chunk(t + 1, st, g(st - 16)))
                    fillers.extend(k_chunk(t + 1, 2, g(-8)))
                    for st in range(8, 12):
                        fillers.extend(v_chunk(t + 1, st, g(st - 16)))
                    fillers.extend(k_chunk(t + 1, 3, g(-4)))
                    for st in range(12, ST):
                        fillers.extend(v_chunk(t + 1, st, g(st - 16)))
                    fillers.extend(q_chunk(t + 1, 1, g(0)))
                    fillers.extend(q_chunk(t + 1, 2, g(16)))
                    fillers.extend(q_chunk(t + 1, 3, g(32)))
                if t == DT - 1 and c >= 1:
                    # SC^T for q-range c-1 is complete across all pairs; gate
                    # two iterations in so its DVE evict lands first
                    for t2 in range(ET):
                        fillers.extend(out_chunk(t2, c - 1, w * ST + 2))
                qh0_pending = t == DT - 1 and c == NCH - 1

                pvh = [
                    ppv.tile(
                        [P, 4 * (HD + 1)], dt.float32, tag="pv", name=f"pv{t}{c}{i}"
                    )
                    for i in range(2)
                ]
                for j in range(ST):
                    sc = sc_next
                    p = ppool.tile([P, 1024], dt.bfloat16, tag="p")
                    nc.scalar.activation(
                        p[:], sc[:], mybir.ActivationFunctionType.Exp, scale=SCALE
                    )
                    # fillers first so produced kt/qt precede dependent scores
                    # in the in-order PE queue (avoids head-of-line blocking).
                    # Budget: generous while draining the pair-0 backlog
                    # (Act is DMA-gated then anyway), just under Act pace in
                    # steady state so the PE never out-runs into Act stalls.
                    if w < 2:
                        budget = 1500
                    elif t == DT - 1 and c == NCH - 1:
                        budget = 1400
                    else:
                        budget = 1250
                    pump(budget, w * ST + j)
                    if j < ST - 1:
                        sc_next = emit_scores(w, j + 1)
                    elif w + 1 < len(windows):
                        sc_next = emit_scores(w + 1, 0)
                    need(("v", t, j))
                    for h_i in range(2):
                        for qs in range(4):
                            nc.tensor.matmul(
                                pvh[h_i][:, qs * (HD + 1) : (qs + 1) * (HD + 1)],
                                p[:, h_i * 512 + qs * P : h_i * 512 + (qs + 1) * P],
                                va[:, j, 2 * t + h_i, :],
                                start=(j == 0 and qs == 0),
                                stop=(j == ST - 1 and qs == 3),
                            )
                # window epilogue: normalize + transpose
                scns = [[None] * 4 for _ in range(2)]
                for h_i in range(2):
                    rr = rrpool.tile([P, 4], dt.float32, tag="rr", name=f"rr{t}{c}{h_i}")
                    nc.vector.reciprocal(
                        rr[:], pvh[h_i][:, HD : 4 * (HD + 1) : HD + 1]
                    )
                    for qs in range(4):
                        scn = scnpool.tile([P, HD], dt.bfloat16, tag="scn")
                        nc.vector.tensor_scalar_mul(
                            scn[:],
                            pvh[h_i][:, qs * (HD + 1) : qs * (HD + 1) + HD],
                            rr[:, qs : qs + 1],
                        )
                        scns[h_i][qs] = scn
                sct_ps = ppv.tile([P, CH], dt.bfloat16, tag="pv", name=f"tp{t}{c}")
                for h_i in range(2):
                    for qs in range(4):
                        nc.tensor.transpose(
                            sct_ps[h_i * HD : (h_i + 1) * HD, qs * P : (qs + 1) * P],
                            scns[h_i][qs][:],
                            ident[:],
                        )
                nc.vector.tensor_copy(sct[:, t, c * CH : (c + 1) * CH], sct_ps[:])
                if qh0_pending:
                    nc.sync.dma_start(
                        out_d[:, 0:1024].rearrange("(t p) q -> p t q", p=P),
                        ev[:, :, 0:1024],
                    )

            # ---------------- tail: leftover qr2, then final q-range ------
            pump(10**9)
            nc.sync.dma_start(
                out_d[:, 1024:1536].rearrange("(t p) q -> p t q", p=P),
                ev[:, :, 1024:1536],
            )
            for t2 in range(ET):
                for _, f, _e, _k in out_chunk(t2, 3):
                    f()
                # ship each e'-tile as soon as it is evicted so the last DMA
                # only covers one tile
                nc.sync.dma_start(
                    out_d[t2 * P : (t2 + 1) * P, 1536:2048],
                    ev[:, t2, 1536:2048],
                )

    nc.compile()
    return nc


def _prep_inputs(x, W_qkv, b_qkv, W_out, b_out):
    """Host-side sharding + layout prep. Returns per-core input maps."""
    w = W_qkv.reshape(E, H, 3, HD)
    wq_f = np.ascontiguousarray(w[:, :, 0, :].reshape(E, E)).astype(_BF16)
    wk_f = np.ascontiguousarray(w[:, :, 1, :].reshape(E, E)).astype(_BF16)
    wv_f = np.ascontiguousarray(w[:, :, 2, :].reshape(E, E)).astype(_BF16)
    b3 = b_qkv.reshape(H, 3, HD)
    bq_f = np.ascontiguousarray(b3[:, 0, :].reshape(E)).astype(np.float32)

    in_maps = []
    xts = [np.ascontiguousarray(x[b].T).astype(_BF16) for b in range(B)]
    halves = []
    for hh in range(2):
        cols = slice(hh * 512, (hh + 1) * 512)
        def pair_major(wf):
            # [E, 512] -> [P, DT, ET, P]: partition = e-within-tile, then
            # (pair, e-tile, col-within-pair) so per-pair slices are contiguous
            return np.ascontiguousarray(
                wf[:, cols].reshape(ET, P, DT, P).transpose(1, 2, 0, 3)
            )

        wq = pair_major(wq_f)
        wk = pair_major(wk_f)
        wv = pair_major(wv_f)
        wo = np.ascontiguousarray(
            W_out[hh * 512 : (hh + 1) * 512, :].reshape(DT, P, E).transpose(1, 0, 2)
        ).astype(_BF16)
        bq = np.ascontiguousarray(bq_f[cols].reshape(DT, P).T)
        halves.append({"wq": wq, "wk": wk, "wv": wv, "wo": wo, "bq": bq})
    for core in range(N_CORES):
        b, hh = core // 2, core % 2
        m = {"xt": xts[b]}
        m.update(halves[hh])
        in_maps.append(m)
    return in_maps


def run_raw(x, W_qkv, b_qkv, W_out, b_out, trace=False, **kw):
    """Run on hardware; returns (full_output [B,S,E] f32, BassKernelResults)."""
    global _cached
    from concourse.bass_utils import run_bass_kernel_spmd

    if _cached is None:
        _cached = _build()
    nc = _cached
    in_maps = _prep_inputs(x, W_qkv, b_qkv, W_out, b_out)
    res = run_bass_kernel_spmd(
        nc, in_maps, core_ids=list(range(N_CORES)), trace=trace, **kw
    )
    # host: sum row-split partials in fp32, transpose, add fused bias row
    bv_f = np.asarray(b_qkv, dtype=np.float64).reshape(H, 3, HD)[:, 2, :].reshape(E)
    bias_row = (
        np.asarray(b_out, dtype=np.float64) + bv_f @ np.asarray(W_out, dtype=np.float64)
    ).astype(np.float32)
    out = np.empty((B, S, E), dtype=np.float32)
    for b in range(B):
        o0 = np.asarray(res.results[2 * b]["out"]).astype(np.float32)
        o1 = np.asarray(res.results[2 * b + 1]["out"]).astype(np.float32)
        out[b] = (o0 + o1).T + bias_row
    return out, res


def kernel(x, W_qkv, b_qkv, W_out, b_out):
    out, _ = run_raw(x, W_qkv, b_qkv, W_out, b_out)
    return out


# revision 24
# speedup vs baseline: 1.6664x; 1.0141x over previous
"""Multi-head attention (B=4, S=2048, E=1024, H=16) on 8 TRN2 NeuronCores.

Sharding: batch x head-half tensor parallel -- core c handles batch c//2,
heads (c%2)*8 .. (c%2)*8+8, for ALL 2048 query rows.  QKV projections are
column-split by head (no redundant work); the output projection is
row-split: each core produces a partial out^T [E, S] (bf16) and the HOST
sums the two partials per batch in fp32 (plus the fused bias row
b_out + b_v @ W_out).

Key layout/efficiency choices (cost model: matmul time = streamed columns):
  - x arrives as x^T [E, S]; Q^T/K^T [d, s] via matmul(lhsT=W_slice, rhs=x^T).
  - scores transposed: S^T[k, q] = matmul(lhsT=K^T tile, rhs=Q^T), streamed
    over q; exp on ScalarE from PSUM (the attention-phase pacer).
  - PV computed NON-transposed: PV[q, d] = matmul(lhsT=p[k, q-tile],
    rhs=V_aug[k, 65]) -- streams only 65 columns per (q-tile, k-tile), half
    the PE work of the transposed form.  The ones column of V_aug yields the
    softmax denominators in column 64; normalization is a per-partition
    tensor_scalar multiply with the reciprocal of the strided denominators.
  - normalized SC[q, d] tiles are transposed back to SC^T via PE transpose
    (identity matmul) for the output projection, which streams q.
  - K bias is dropped entirely (softmax-invariant); V bias and out bias are
    folded into a host-side bias row (b_out + b_v @ W_out) added after the
    partial sum; only the Q bias is applied in-kernel.
  - score matmuls are software-pipelined one j-step ahead (across window
    boundaries too) so the Activation engine never waits; "filler"
    projection / output matmuls are column-budget pumped into each j
    iteration, deadline-ordered against the progressive x^T DMA arrival,
    so the PE never idles while Activation catches up.

Compute bf16, fp32 PSUM accumulation.
"""

import sys

if "/opt/trn_rl_repo" not in sys.path:
    sys.path.insert(0, "/opt/trn_rl_repo")

import numpy as np
import ml_dtypes

B, S, E, H = 4, 2048, 1024, 16
HD = E // H  # 64
N_CORES = 8
P = 128
ET = E // P          # 8 e-tiles (contraction tiles for projections)
ST = S // P          # 16 s-tiles (k-tiles)
HPC = 8              # heads per core
DT = HPC * HD // P   # 4 d-tiles per core (2 heads per tile)
NCH = 4              # q-chunks of 512
CH = 512

_BF16 = ml_dtypes.bfloat16

_cached = None

SCALE = 1.0 / float(np.sqrt(HD))


def _build():
    import concourse.bass as bass
    import concourse.tile as tile
    import concourse.mybir as mybir
    from concourse import bacc
    from concourse.masks import make_identity
    from collections import deque

    dt = mybir.dt
    nc = bacc.Bacc("TRN2", target_bir_lowering=False, debug=False)

    xt_d = nc.dram_tensor("xt", [E, S], dt.bfloat16, kind="ExternalInput").ap()
    wq_d = nc.dram_tensor("wq", [P, DT, ET, P], dt.bfloat16, kind="ExternalInput").ap()
    wk_d = nc.dram_tensor("wk", [P, DT, ET, P], dt.bfloat16, kind="ExternalInput").ap()
    wv_d = nc.dram_tensor("wv", [P, DT, ET, P], dt.bfloat16, kind="ExternalInput").ap()
    wo_d = nc.dram_tensor("wo", [P, DT, E], dt.bfloat16, kind="ExternalInput").ap()
    bq_d = nc.dram_tensor("bq", [P, DT], dt.float32, kind="ExternalInput").ap()
    out_d = nc.dram_tensor("out", [E, S], dt.bfloat16, kind="ExternalOutput").ap()

    with tile.TileContext(nc) as tc:
        with (
            tc.tile_pool(name="const", bufs=1) as cpool,
            tc.tile_pool(name="acts", bufs=1) as apool,
            tc.tile_pool(name="p", bufs=6) as ppool,
            tc.tile_pool(name="scn", bufs=10) as scnpool,
            tc.tile_pool(name="rr", bufs=4) as rrpool,
            tc.tile_pool(name="psc", bufs=2, space="PSUM") as psc,
            tc.tile_pool(name="ppv", bufs=2, space="PSUM") as ppv,
            tc.tile_pool(name="pproj", bufs=2, space="PSUM") as pproj,
        ):
            # ---------------- constants / DMAs ----------------
            xt = cpool.tile([P, ET, S], dt.bfloat16)
            wq = cpool.tile([P, DT, ET, P], dt.bfloat16)
            wk = cpool.tile([P, DT, ET, P], dt.bfloat16)
            wv = cpool.tile([P, DT, ET, P], dt.bfloat16)
            wo = cpool.tile([P, DT, E], dt.bfloat16)
            bq = cpool.tile([P, DT], dt.float32)
            ident = cpool.tile([P, P], dt.bfloat16)

            # DMA schedule: the startup critical path is this serial chain,
            # so pair-0's weight slices (contiguous in the pair-major DRAM
            # layout, ~0.7us each) and the first half of xt chunk 0 come
            # first; the rest follows in deadline order; wo is needed last.
            def xt_cols_dma(lo, hi):
                nc.sync.dma_start(
                    xt[:, :, lo:hi],
                    xt_d[:, lo:hi].rearrange("(eo p) s -> p eo s", p=P),
                )

            nc.sync.dma_start(wq[:, 0], wq_d[:, 0])
            xt_cols_dma(0, 256)
            nc.sync.dma_start(wk[:, 0], wk_d[:, 0])
            xt_cols_dma(256, 512)
            nc.sync.dma_start(bq[:], bq_d)
            nc.sync.dma_start(wv[:, 0], wv_d[:, 0])
            for m in range(2, 8):
                xt_cols_dma(m * 256, (m + 1) * 256)
            nc.sync.dma_start(wq[:, 1:], wq_d[:, 1:])
            nc.sync.dma_start(wk[:, 1:], wk_d[:, 1:])
            nc.sync.dma_start(wv[:, 1:], wv_d[:, 1:])
            nc.sync.dma_start(wo[:], wo_d)
            make_identity(nc, ident[:])

            qt = apool.tile([P, DT, S], dt.bfloat16)   # Q^T + bq
            kt = apool.tile([P, DT, S], dt.bfloat16)   # K^T (no bias)
            va = apool.tile([P, ST, HPC, HD + 1], dt.bfloat16)  # V | ones col
            sct = apool.tile([P, DT, S], dt.bfloat16)  # normalized SC^T
            ev = apool.tile([P, ET, S], dt.bfloat16)   # staged out^T partial
            evp = apool.tile([P, ET, CH], dt.float32)  # qr3 ct0+ct1 partial

            nc.gpsimd.memset(va[:, :, :, HD : HD + 1], 1.0)

            # ---------------- filler machinery ----------------
            # Fillers are (cols, callable, earliest) triples, each emitting
            # ONE PE matmul (chunk evictions are emitted inline by the
            # chunk's last matmul; they run on DVE and cost no PE time).
            # pump(budget, now) pops fillers until the column budget is
            # spent, but never emits a filler before its `earliest`
            # window*16+j position -- this keeps DMA-gated matmuls from
            # entering the in-order PE queue early and head-of-line blocking
            # the attention stream behind them.
            fillers = deque()
            done_keys = set()

            def _pop_one():
                cols, fn, _e, key = fillers.popleft()
                fn()
                if key is not None:
                    done_keys.add(key)
                return cols

            def need(*keys):
                # correctness guard: force-emit queued fillers (in order,
                # ignoring gates/budgets) until every producer key a consumer
                # is about to read has been emitted.  Emission order is what
                # the tile framework builds dependencies from.
                for key in keys:
                    while key not in done_keys:
                        assert fillers, f"missing producer for {key}"
                        _pop_one()

            def q_chunk(t, c, earliest=0):
                state = {}
                key = ("q", t, c)

                def mk(e):
                    def emit():
                        if e == 0:
                            state["ps"] = pproj.tile(
                                [P, CH], dt.float32, tag="proj", name=f"q{t}{c}"
                            )
                        nc.tensor.matmul(
                            state["ps"][:],
                            wq[:, t, e, :],
                            xt[:, e, c * CH : (c + 1) * CH],
                            start=(e == 0),
                            stop=(e == ET - 1),
                        )
                        if e == ET - 1:
                            nc.vector.tensor_scalar_add(
                                qt[:, t, c * CH : (c + 1) * CH],
                                state["ps"][:],
                                bq[:, t : t + 1],
                            )

                    return emit

                return [(CH, mk(e), earliest, key if e == ET - 1 else None) for e in range(ET)]

            def k_chunk(t, c, earliest=0):
                state = {}
                key = ("k", t, c)

                def mk(e):
                    def emit():
                        if e == 0:
                            state["ps"] = pproj.tile(
                                [P, CH], dt.float32, tag="proj", name=f"k{t}{c}"
                            )
                        nc.tensor.matmul(
                            state["ps"][:],
                            wk[:, t, e, :],
                            xt[:, e, c * CH : (c + 1) * CH],
                            start=(e == 0),
                            stop=(e == ET - 1),
                        )
                        if e == ET - 1:
                            nc.vector.tensor_copy(
                                kt[:, t, c * CH : (c + 1) * CH], state["ps"][:]
                            )

                    return emit

                return [(CH, mk(e), earliest, key if e == ET - 1 else None) for e in range(ET)]

            def k_tile(t, j, earliest=0):
                # fine-grained K chunk: a single 128-wide k-tile
                state = {}
                key = ("ktile", t, j)

                def mk(e):
                    def emit():
                        if e == 0:
                            state["ps"] = pproj.tile(
                                [P, P], dt.float32, tag="proj", name=f"kt{t}_{j}"
                            )
                        nc.tensor.matmul(
                            state["ps"][:],
                            wk[:, t, e, :],
                            xt[:, e, j * P : (j + 1) * P],
                            start=(e == 0),
                            stop=(e == ET - 1),
                        )
                        if e == ET - 1:
                            nc.vector.tensor_copy(
                                kt[:, t, j * P : (j + 1) * P], state["ps"][:]
                            )

                    return emit

                return [(P, mk(e), earliest, key if e == ET - 1 else None) for e in range(ET)]

            def q_half(t, c, half, earliest=0, width=256):
                # narrow Q chunk (startup / de-clumping granularity)
                lo = c * CH + half * width
                state = {}
                key = ("qhalf", t, c, half)

                def mk(e):
                    def emit():
                        if e == 0:
                            state["ps"] = pproj.tile(
                                [P, width], dt.float32, tag="proj", name=f"qh{t}{c}{half}"
                            )
                        nc.tensor.matmul(
                            state["ps"][:],
                            wq[:, t, e, :],
                            xt[:, e, lo : lo + width],
                            start=(e == 0),
                            stop=(e == ET - 1),
                        )
                        if e == ET - 1:
                            nc.vector.tensor_scalar_add(
                                qt[:, t, lo : lo + width],
                                state["ps"][:],
                                bq[:, t : t + 1],
                            )

                    return emit

                return [
                    (width, mk(e), earliest, key if e == ET - 1 else None)
                    for e in range(ET)
                ]

            def v_chunk(t, st, earliest=0):
                state = {}
                key = ("v", t, st)

                def mk(e):
                    def emit():
                        if e == 0:
                            state["ps"] = pproj.tile(
                                [P, 2 * HD], dt.float32, tag="proj", name=f"v{t}{st}"
                            )
                        nc.tensor.matmul(
                            state["ps"][:],
                            xt[:, e, st * P : (st + 1) * P],
                            wv[:, t, e, :],
                            start=(e == 0),
                            stop=(e == ET - 1),
                        )
                        if e == ET - 1:
                            nc.vector.tensor_copy(
                                va[:, st, 2 * t : 2 * t + 2, 0:HD],
                                state["ps"].rearrange("p (h d) -> p h d", d=HD),
                            )

                    return emit

                return [(2 * HD, mk(e), earliest, key if e == ET - 1 else None) for e in range(ET)]

            def out_chunk(t2, qr, earliest=0):
                state = {}

                def mk(ct):
                    def emit():
                        if ct == 0:
                            state["ps"] = pproj.tile(
                                [P, CH], dt.float32, tag="proj", name=f"o{t2}{qr}"
                            )
                        nc.tensor.matmul(
                            state["ps"][:],
                            wo[:, ct, t2 * P : (t2 + 1) * P],
                            sct[:, ct, qr * CH : (qr + 1) * CH],
                            start=(ct == 0),
                            stop=(ct == DT - 1),
                        )
                        if ct == DT - 1:
                            nc.vector.tensor_copy(
                                ev[:, t2, qr * CH : (qr + 1) * CH], state["ps"][:]
                            )
                            if qr == 2:
                                # ship immediately: the DMA engines are idle
                                # during the last window, and this keeps the
                                # final q-range's DMAs off the tail
                                nc.sync.dma_start(
                                    out_d[
                                        t2 * P : (t2 + 1) * P, qr * CH : (qr + 1) * CH
                                    ],
                                    ev[:, t2, qr * CH : (qr + 1) * CH],
                                )

                    return emit

                return [(CH, mk(ct), earliest, None) for ct in range(DT)]

            def o3p_chunk(t2, earliest=0):
                # first half (ct 0,1) of the final q-range's output chunk,
                # computed during pair-2's slack and staged to SBUF so the
                # post-attention tail only runs ct 2,3 plus a fused add
                state = {}

                def mk(ct):
                    def emit():
                        if ct == 0:
                            state["ps"] = pproj.tile(
                                [P, CH], dt.float32, tag="proj", name=f"o3p{t2}"
                            )
                        nc.tensor.matmul(
                            state["ps"][:],
                            wo[:, ct, t2 * P : (t2 + 1) * P],
                            sct[:, ct, 3 * CH : 4 * CH],
                            start=(ct == 0),
                            stop=(ct == 1),
                        )
                        if ct == 1:
                            nc.vector.tensor_copy(evp[:, t2, :], state["ps"][:])

                    return emit

                return [(CH, mk(ct), earliest, None) for ct in range(2)]

            def pump(budget, now=10**9):
                while fillers and budget > 0 and fillers[0][2] <= now:
                    budget -= _pop_one()

            def k_cover_key(t, j):
                # pair 0 produces all k-tiles individually, chunks for others
                if t == 0:
                    return ("ktile", t, j)
                return ("k", t, j // 4)

            # ---------------- pipelined attention windows ----------------
            windows = [(t, c) for t in range(DT) for c in range(NCH)]

            q_by_halves = {(0, 0), (0, 1), (0, 2), (0, 3)}

            def q_cover(t, c):
                if (t, c) in q_by_halves:
                    return (("qhalf", t, c, 0), ("qhalf", t, c, 1))
                return (("q", t, c),)

            def emit_scores(w, j):
                t, c = windows[w]
                need(k_cover_key(t, j), *q_cover(t, j // 10**9 if False else c))
                sc = psc.tile([P, 1024], dt.float32, tag="sc", name=f"sc{t}{c}{j}")
                nc.tensor.matmul(
                    sc[:, 0:512],
                    kt[0:HD, t, j * P : (j + 1) * P],
                    qt[0:HD, t, c * CH : (c + 1) * CH],
                    start=True,
                    stop=True,
                )
                nc.tensor.matmul(
                    sc[:, 512:1024],
                    kt[HD:P, t, j * P : (j + 1) * P],
                    qt[HD:P, t, c * CH : (c + 1) * CH],
                    start=True,
                    stop=True,
                )
                return sc

            # preamble: K tile j0 (smallest work unblocking the first score
            # matmul), Q(0,c0) in two 256-halves matching the split xt DMA,
            # then the first score tile.
            for group in (q_half(0, 0, 0), k_tile(0, 0), q_half(0, 0, 1)):
                for _, f, _e, key in group:
                    f()
                    if key is not None:
                        done_keys.add(key)
            done_keys.add(("q", 0, 0))
            # pair-0 fillers, ordered by earliest-emission gate (a gated head
            # blocks the whole queue): K tile j is needed by score j, K
            # chunk c by scores 4c.. (xt chunk c DMA), V s-tile st by PV(st)
            # (wv + xt chunk st//4 DMA), Q chunk c by window (0, c).
            for j in range(1, ST):
                gate = max(0, j - 3)
                fillers.extend(k_tile(0, j, gate))
                fillers.extend(v_chunk(0, j - 1, gate))
                if j == 7:
                    fillers.extend(q_half(0, 1, 0, 5))
                elif j == 9:
                    fillers.extend(q_half(0, 1, 1, 7))
            fillers.extend(v_chunk(0, ST - 1, 13))
            fillers.extend(q_half(0, 2, 0, 20))
            fillers.extend(q_half(0, 2, 1, 23))
            fillers.extend(q_half(0, 3, 0, 36))
            fillers.extend(q_half(0, 3, 1, 39))

            sc_next = emit_scores(0, 0)

            for w, (t, c) in enumerate(windows):
                # window prologue: queue fillers
                if c == 0 and t + 1 < DT:
                    # next pair's projections, gated just-in-time (one window
                    # of margin before each chunk's first consumer) so the
                    # filler stream stays flat instead of bursting into the
                    # early windows and stalling the Activation engine.
                    # Pair-1 additionally waits for the rest-weight DMAs.
                    base = (t + 1) * 4 * ST
                    floor = w * ST + (20 if t == 0 else 0)

                    def g(rel):
                        return max(base + rel, floor)

                    fillers.extend(q_chunk(t + 1, 0, g(-16)))
                    fillers.extend(k_chunk(t + 1, 0, g(-16)))
                    for st in range(4):
                        fillers.extend(v_chunk(t + 1, st, g(st - 16)))
                    fillers.extend(k_chunk(t + 1, 1, g(-12)))
                    for st in range(4, 8):
                        fillers.extend(v_chunk(t + 1, st, g(st - 16)))
                    fillers.extend(k_chunk(t + 1, 2, g(-8)))
                    for st in range(8, 12):
                        fillers.extend(v_chunk(t + 1, st, g(st - 16)))
                    fillers.extend(k_chunk(t + 1, 3, g(-4)))
                    for st in range(12, ST):
                        fillers.extend(v_chunk(t + 1, st, g(st - 16)))
                    fillers.extend(q_chunk(t + 1, 1, g(0)))
                    fillers.extend(q_chunk(t + 1, 2, g(16)))
                    fillers.extend(q_chunk(t + 1, 3, g(32)))
                if t == 2 and c == 0:
                    # qr3 ct0/ct1 partials: pairs 0-1's SC^T for the final
                    # q-range are complete; spread across pair-2's windows
                    for t2 in range(ET):
                        fillers.extend(o3p_chunk(t2, w * ST + 2 + t2 * 6))
                if t == DT - 1 and c >= 1:
                    # SC^T for q-range c-1 is complete across all pairs; gate
                    # a few iterations in so its transpose lands first
                    for t2 in range(ET):
                        fillers.extend(out_chunk(t2, c - 1, w * ST + 3))
                qh0_pending = t == DT - 1 and c == NCH - 1

                pvh = [
                    ppv.tile(
                        [P, 4 * (HD + 1)], dt.float32, tag="pv", name=f"pv{t}{c}{i}"
                    )
                    for i in range(2)
                ]
                for j in range(ST):
                    sc = sc_next
                    p = ppool.tile([P, 1024], dt.bfloat16, tag="p")
                    nc.scalar.activation(
                        p[:], sc[:], mybir.ActivationFunctionType.Exp, scale=SCALE
                    )
                    # fillers first so produced kt/qt precede dependent scores
                    # in the in-order PE queue (avoids head-of-line blocking).
                    # Budget: generous while draining the pair-0 backlog
                    # (Act is DMA-gated then anyway), just under Act pace in
                    # steady state so the PE never out-runs into Act stalls.
                    if w < 2:
                        budget = 1500
                    elif t == DT - 1 and c == NCH - 1:
                        budget = 1400
                    else:
                        budget = 1250
                    pump(budget, w * ST + j)
                    if j < ST - 1:
                        sc_next = emit_scores(w, j + 1)
                    elif w + 1 < len(windows):
                        sc_next = emit_scores(w + 1, 0)
                    need(("v", t, j))
                    for h_i in range(2):
                        for qs in range(4):
                            nc.tensor.matmul(
                                pvh[h_i][:, qs * (HD + 1) : (qs + 1) * (HD + 1)],
                                p[:, h_i * 512 + qs * P : h_i * 512 + (qs + 1) * P],
                                va[:, j, 2 * t + h_i, :],
                                start=(j == 0 and qs == 0),
                                stop=(j == ST - 1 and qs == 3),
                            )
                # window epilogue: normalize into combined [q, (head, d)]
                # tiles, then transpose into SC^T -- via the DMA xbar for most
                # windows (off the PE), via PE-transpose for the last window
                # (the tail is latency-bound on this sct landing).
                scn2s = [
                    scnpool.tile([P, 2, HD], dt.bfloat16, tag="scn", name=f"sn{w}_{i}")
                    for i in range(4)
                ]
                for h_i in range(2):
                    rr = rrpool.tile([P, 4], dt.float32, tag="rr", name=f"rr{t}{c}{h_i}")
                    nc.vector.reciprocal(
                        rr[:], pvh[h_i][:, HD : 4 * (HD + 1) : HD + 1]
                    )
                    for qs in range(4):
                        nc.vector.tensor_scalar_mul(
                            scn2s[qs][:, h_i, :],
                            pvh[h_i][:, qs * (HD + 1) : qs * (HD + 1) + HD],
                            rr[:, qs : qs + 1],
                        )
                if w < len(windows) - 1:
                    for qs in range(4):
                        nc.sync.dma_start_transpose(
                            sct[:, t, c * CH + qs * P : c * CH + (qs + 1) * P],
                            scn2s[qs][:],
                        )
                else:
                    sct_ps = ppv.tile([P, CH], dt.bfloat16, tag="pv", name=f"tp{t}{c}")
                    for h_i in range(2):
                        for qs in range(4):
                            nc.tensor.transpose(
                                sct_ps[
                                    h_i * HD : (h_i + 1) * HD, qs * P : (qs + 1) * P
                                ],
                                scn2s[qs][:, h_i, :],
                                ident[:],
                            )
                    nc.vector.tensor_copy(
                        sct[:, t, c * CH : (c + 1) * CH], sct_ps[:]
                    )
                if qh0_pending:
                    nc.sync.dma_start(
                        out_d[:, 0:1024].rearrange("(t p) q -> p t q", p=P),
                        ev[:, :, 0:1024],
                    )

            # ---------------- tail: leftover qr2, then final q-range ------
            pump(10**9)
            for t2 in range(ET):
                ps = pproj.tile([P, CH], dt.float32, tag="proj", name=f"o3t{t2}")
                for ct in (2, 3):
                    nc.tensor.matmul(
                        ps[:],
                        wo[:, ct, t2 * P : (t2 + 1) * P],
                        sct[:, ct, 3 * CH : 4 * CH],
                        start=(ct == 2),
                        stop=(ct == 3),
                    )
                nc.vector.tensor_tensor(
                    ev[:, t2, 1536:2048], ps[:], evp[:, t2, :], mybir.AluOpType.add
                )
                # ship each e'-tile as it completes
                nc.sync.dma_start(
                    out_d[t2 * P : (t2 + 1) * P, 1536:2048],
                    ev[:, t2, 1536:2048],
                )

    nc.compile()
    return nc


def _prep_inputs(x, W_qkv, b_qkv, W_out, b_out):
    """Host-side sharding + layout prep. Returns per-core input maps."""
    w = W_qkv.reshape(E, H, 3, HD)
    wq_f = np.ascontiguousarray(w[:, :, 0, :].reshape(E, E)).astype(_BF16)
    wk_f = np.ascontiguousarray(w[:, :, 1, :].reshape(E, E)).astype(_BF16)
    wv_f = np.ascontiguousarray(w[:, :, 2, :].reshape(E, E)).astype(_BF16)
    b3 = b_qkv.reshape(H, 3, HD)
    bq_f = np.ascontiguousarray(b3[:, 0, :].reshape(E)).astype(np.float32)

    in_maps = []
    xts = [np.ascontiguousarray(x[b].T).astype(_BF16) for b in range(B)]
    halves = []
    for hh in range(2):
        cols = slice(hh * 512, (hh + 1) * 512)
        def pair_major(wf):
            # [E, 512] -> [P, DT, ET, P]: partition = e-within-tile, then
            # (pair, e-tile, col-within-pair) so per-pair slices are contiguous
            return np.ascontiguousarray(
                wf[:, cols].reshape(ET, P, DT, P).transpose(1, 2, 0, 3)
            )

        wq = pair_major(wq_f)
        wk = pair_major(wk_f)
        wv = pair_major(wv_f)
        wo = np.ascontiguousarray(
            W_out[hh * 512 : (hh + 1) * 512, :].reshape(DT, P, E).transpose(1, 0, 2)
        ).astype(_BF16)
        bq = np.ascontiguousarray(bq_f[cols].reshape(DT, P).T)
        halves.append({"wq": wq, "wk": wk, "wv": wv, "wo": wo, "bq": bq})
    for core in range(N_CORES):
        b, hh = core // 2, core % 2
        m = {"xt": xts[b]}
        m.update(halves[hh])
        in_maps.append(m)
    return in_maps


def run_raw(x, W_qkv, b_qkv, W_out, b_out, trace=False, **kw):
    """Run on hardware; returns (full_output [B,S,E] f32, BassKernelResults)."""
    global _cached
    from concourse.bass_utils import run_bass_kernel_spmd

    if _cached is None:
        _cached = _build()
    nc = _cached
    in_maps = _prep_inputs(x, W_qkv, b_qkv, W_out, b_out)
    res = run_bass_kernel_spmd(
        nc, in_maps, core_ids=list(range(N_CORES)), trace=trace, **kw
    )
    # host: sum row-split partials in fp32, transpose, add fused bias row
    bv_f = np.asarray(b_qkv, dtype=np.float64).reshape(H, 3, HD)[:, 2, :].reshape(E)
    bias_row = (
        np.asarray(b_out, dtype=np.float64) + bv_f @ np.asarray(W_out, dtype=np.float64)
    ).astype(np.float32)
    out = np.empty((B, S, E), dtype=np.float32)
    for b in range(B):
        o0 = np.asarray(res.results[2 * b]["out"]).astype(np.float32)
        o1 = np.asarray(res.results[2 * b + 1]["out"]).astype(np.float32)
        out[b] = (o0 + o1).T + bias_row
    return out, res


def kernel(x, W_qkv, b_qkv, W_out, b_out):
    out, _ = run_raw(x, W_qkv, b_qkv, W_out, b_out)
    return out
